# revision 9
# baseline (speedup 1.0000x reference)
"""Trainium2 Bass kernel for the cross-head MultiHeadAttention module.

Reference computation (per row r of x flattened to (N*L, E)):
    q = x @ Wq; k = x @ Wk; v = x @ Wv           (E = 1024, H = 16, D = 64)
    energy[r, i, j] = sum_d q[r,i,d] * k[r,j,d]  (cross-head, per position)
    attn = softmax(energy / 32, axis=j)
    out[r, i, :] = sum_j attn[r,i,j] * v[r,j,:]
    y = out.reshape(R, E) @ Wo + bo

Distribution: data-parallel over rows (N*L = 16384 -> 2048 rows/core x 8).

Per-core design (all big matmuls in bf16 on the PE array), v5:
  *  Q/K projections run transposed (features on partitions, rows free).
     Their attention-layout rebuild (qd2b/kht2 block-diagonal slabs) is a
     pure partition-shift, so it runs as 32 direct SBUF->SBUF DMAs per
     tensor (512B runs both sides) with NO DRAM round trip; each DMA only
     depends on one projection chunk's psum->sbuf copy, so the rebuild
     streams behind the projection instead of after it.
  *  V runs natural (rows on partitions) and still round-trips through
     DRAM (its rebuild needs a partition<->free exchange which SBUF APs
     cannot express).  vd is double-buffered by pass parity so the
     readback issues immediately after the stage-out instead of waiting
     for the previous pass's AV matmuls (WAR removed).
  *  Energy: ONE matmul per row pair (pi, pi+RC/2): lhsT = qd2b[:, :, pi]
     ([128, 32] block-diagonal), rhs = kht2[:, :, pi] ([128, 16]); out is
     a dense [32, 16] block of a 256-row psum bank so softmax runs on
     dense [128, 512] tiles.
  *  softmax: exp (no max-subtraction: energies ~N(0, 1/16)), row-sum,
     reciprocal, scale+cast-to-bf16, 32x32-block vector transpose.
  *  A@V: ONE matmul per row pair; avp psum is 2 double-buffered 2-bank
     tiles (b-halves) so bank1's matmuls only wait on bank0's first-half
     extraction.
  *  y^T: full-width Wo matmuls accumulated in psum; + bo; DMA out.
  *  Schedule: the tensor queue interleaves pass p-1's attention with
     pass p's projections at chunk granularity:
       projq(p) | energy0(p-1) | projk(p) | energy1(p-1) | av0(p-1) |
       projv(p)+vd-readback(p) | av1(p-1) | Wo(p-1)
     so softmax / extraction / readback chains hide under projection
     matmuls and the PE p-state stays high.  Startup DMAs are split
     per-128-column chunk and ordered xtc -> Wq -> Wk -> Wv -> Wo.
"""

import numpy as np
import ml_dtypes

import concourse.bass as bass
from concourse import bacc
import concourse.tile as tile
from concourse import mybir
from concourse.bass_utils import run_bass_kernel_spmd

F32 = mybir.dt.float32
BF16 = mybir.dt.bfloat16
AF = mybir.ActivationFunctionType
ALU = mybir.AluOpType
AX = mybir.AxisListType

E = 1024
H = 16
D = 64
NCORE = 8


def build_nc(R, RC):
    """Per-core kernel program: R rows total, processed in passes of RC."""
    NP = R // RC          # passes
    NBK = RC // 256       # dense energy banks per pass (256 rows each)
    PH = RC // 2          # row pairs per pass

    nc = bacc.Bacc("TRN2", target_bir_lowering=False, debug=False)

    xt = nc.dram_tensor("xt", [E, R], BF16, kind="ExternalInput")
    wq = nc.dram_tensor("wq", [E, E], BF16, kind="ExternalInput")
    wk = nc.dram_tensor("wk", [E, E], BF16, kind="ExternalInput")
    wv = nc.dram_tensor("wv", [E, E], BF16, kind="ExternalInput")
    wo = nc.dram_tensor("wo", [E, E], BF16, kind="ExternalInput")
    yt = nc.dram_tensor("yt", [E, R], F32, kind="ExternalOutput")

    with tile.TileContext(nc) as tc:
        with (
            tc.tile_pool(name="wpool", bufs=1) as wpool,      # persistent
            tc.tile_pool(name="xpool", bufs=2) as xpool,      # xt chunks
            tc.tile_pool(name="spool", bufs=1) as spool,      # q/k/v staging
            tc.tile_pool(name="apool", bufs=2) as apool,      # softmax temps
            tc.tile_pool(name="opool", bufs=1) as opool,      # oft2
            tc.tile_pool(name="ypool", bufs=2) as ypool,      # y staging
            tc.tile_pool(name="dram", bufs=2, space="DRAM") as dpool,
            tc.tile_pool(name="pproj", bufs=2, space="PSUM") as pproj,
            tc.tile_pool(name="pe", bufs=2, space="PSUM") as pe_pool,
            tc.tile_pool(name="pav", bufs=2, space="PSUM") as pav,
        ):
            # ---- persistent loads (split per 128-col chunk; xtc(0) + wq
            # first so the first projection starts as early as possible).
            # DMA traffic classes: latency-critical relayouts (qk, vd) +
            # xtc go on the SP HWDGE (nc.sync); bulk streams (weights,
            # V stage-out, y out) go on the Activation HWDGE (nc.scalar)
            # so bulk descriptor storms never sit ahead of critical ones.
            wq_sb = wpool.tile([128, 8, E], BF16, tag="wq")
            wk_sb = wpool.tile([128, 8, E], BF16, tag="wk")
            wv_sb = wpool.tile([128, 8, E], BF16, tag="wv")
            wo_sb = wpool.tile([128, 8, E], BF16, tag="wo")

            xtr = xt.rearrange("(c p) r -> p c r", p=128)
            xtc0 = xpool.tile([128, 8, RC], BF16, tag="xtc")
            for c in range(8):
                nc.sync.dma_start(xtc0[:, c, :], xtr[:, c, 0:RC])
            for w_sb, w_d in ((wq_sb, wq), (wk_sb, wk), (wv_sb, wv),
                              (wo_sb, wo)):
                wr = w_d.rearrange("(c p) e -> p c e", p=128)
                for c in range(8):
                    nc.scalar.dma_start(w_sb[:, c, :], wr[:, c, :])

            # block-diagonal operand tiles: zero blocks are memset once and
            # never rewritten (per-pass DMAs touch only the data blocks).
            # qd2b/kht2/vd ping-pong per pass parity so pass p+1's rebuild
            # DMAs never wait on pass p's attention matmuls.
            qd2bs, kht2s, vds = [], [], []
            for pp in range(2):
                qd2b_ = wpool.tile([128, 32, PH], BF16, tag=f"qd2b{pp}",
                                   name=f"qd2b{pp}")
                nc.vector.memset(qd2b_[0:64, 16:32, :], 0.0)
                nc.vector.memset(qd2b_[64:128, 0:16, :], 0.0)
                qd2bs.append(qd2b_)
                kht2_ = wpool.tile([128, 16, PH], BF16, tag=f"kht2{pp}",
                                   name=f"kht2{pp}")
                kht2s.append(kht2_)
                vd_ = wpool.tile([128, NBK * 32, 128], BF16, tag=f"vd{pp}",
                                 name=f"vd{pp}")
                nc.vector.memset(vd_[:], 0.0)
                vds.append(vd_)

            oft2 = opool.tile([128, 8, RC], BF16, tag="oft2")

            def proj_tr(name, w_sb, xtc):
                """Transposed projection (features on partitions)."""
                stg = spool.tile([128, 8, RC], BF16, tag=f"stg_{name}")
                for et in range(8):
                    ps = pproj.tile([128, RC], F32, tag="proj")
                    for c in range(8):
                        nc.tensor.matmul(
                            ps[:],
                            w_sb[:, c, et * 128:(et + 1) * 128],
                            xtc[:, c, :],
                            start=(c == 0),
                            stop=(c == 7),
                        )
                    if et % 2 == 0:
                        nc.vector.tensor_copy(stg[:, et, :], ps[:])
                    else:
                        nc.scalar.copy(stg[:, et, :], ps[:])
                return stg

            def relayout_q(it, stg):
                """stg_q -> qd2b[it%2] via 32 partition-shift SBUF DMAs.

                qd2b[64h'+d, 16h'+q, pi] = q^T[d, head q, row 256h'+pi]
                and stg[64(q%2)+d, q//2, r] = q^T[d, head q, row r].
                """
                qd2b_w = qd2bs[it % 2]
                for hp in range(2):
                    for q in range(H):
                        nc.sync.dma_start(
                            qd2b_w[64 * hp:64 * hp + 64, 16 * hp + q, :],
                            stg[64 * (q % 2):64 * (q % 2) + 64, q // 2,
                                256 * hp:256 * hp + 256],
                        )

            def relayout_k(it, stg):
                """stg_k -> kht2[it%2]: kht2[64h'+d, q, pi] = k^T[d, q,
                row 256h'+pi]."""
                kht2_w = kht2s[it % 2]
                for hp in range(2):
                    for q in range(H):
                        nc.sync.dma_start(
                            kht2_w[64 * hp:64 * hp + 64, q, :],
                            stg[64 * (q % 2):64 * (q % 2) + 64, q // 2,
                                256 * hp:256 * hp + 256],
                        )

            def proj_v(it, xtc):
                """Natural (row-major) V projection + DRAM round trip into
                vd[it%2].  Staged and read back per 128-row chunk so the
                readback starts ~3/4 of a projection earlier: chunk rc_
                holds exactly the rows with (h, B) = divmod(rc_, NBK)."""
                vstg = spool.tile([128, RC // 128, E], BF16, tag="stg_v")
                v2d = dpool.tile([RC, E], BF16, tag="dram_v")
                v2dr = v2d[:].rearrange("(rc p) e -> p rc e", p=128)
                vd_w = vds[it % 2]
                vsrc = v2d[:].rearrange(
                    "(h B w b m) (j d) -> h w B b j m d",
                    h=2, B=NBK, w=2, b=4, m=16, j=16,
                )
                for rc_ in range(RC // 128):
                    for h2 in range(2):
                        ps = pproj.tile([128, 512], F32, tag="proj")
                        for c in range(8):
                            nc.tensor.matmul(
                                ps[:],
                                xtc[:, c, rc_ * 128:(rc_ + 1) * 128],
                                wv_sb[:, c, h2 * 512:(h2 + 1) * 512],
                                start=(c == 0),
                                stop=(c == 7),
                            )
                        if h2 == 0:
                            nc.vector.tensor_copy(vstg[:, rc_, 0:512], ps[:])
                        else:
                            nc.scalar.copy(vstg[:, rc_, 512:1024], ps[:])
                    nc.scalar.dma_start(v2dr[:, rc_, :], vstg[:, rc_, :])
                    rho, B = divmod(rc_, NBK)
                    for w in range(2):
                        for b in range(4):
                            nc.sync.dma_start(
                                vd_w[32 * b + 16 * w:
                                     32 * b + 16 * w + 16,
                                     32 * B + rho:32 * B + 32:2,
                                     64 * w:64 * w + 64],
                                vsrc[rho, w, B, b],
                            )

            def energy_softmax(B, par):
                """One dense 256-row energy bank + its softmax; returns att."""
                qd2b, kht2 = qd2bs[par], kht2s[par]
                ep = pe_pool.tile([128, 32, 16], F32, tag="ep")
                for lam in range(128):
                    pi = 128 * B + lam
                    b, s = (lam // 16) % 4, 2 * (lam % 16) + lam // 64
                    nc.tensor.matmul(
                        ep[32 * b:32 * b + 32, s, :],
                        qd2b[:, :, pi],
                        kht2[:, :, pi],
                        start=True,
                        stop=True,
                        tile_position=(0, 32 * b),
                    )
                ex = apool.tile([128, 32, 16], F32, tag="ex")
                nc.scalar.activation(ex[:], ep[:], AF.Exp)
                sm = apool.tile([128, 32], F32, tag="sm")
                nc.vector.reduce_sum(sm[:], ex[:], axis=AX.X)
                rcp = apool.tile([128, 32], F32, tag="rcp")
                nc.vector.reciprocal(rcp[:], sm[:])
                at = apool.tile([128, 32, 16], BF16, tag="at")
                nc.vector.tensor_tensor(
                    at[:], ex[:],
                    rcp[:, :, None].to_broadcast([128, 32, 16]),
                    ALU.mult,
                )
                att = apool.tile([128, 512], BF16, tag="att")
                nc.vector.transpose(att[:], at[:].rearrange("p a b -> p (a b)"))
                return att

            def av_extract(B, att, par):
                """A@V for bank B: two 2-bank psum tiles (b-halves) + 8
                merged extraction copies."""
                vd = vds[par]
                dstx = oft2[:].rearrange(
                    "p g (h Bk wc) -> p g h Bk wc", h=2, Bk=NBK)
                for bh in range(2):
                    avp = pav.tile([128, 2, 32, 16], F32, tag="avp")
                    for b2 in range(2):
                        b = 2 * bh + b2
                        for t in range(32):
                            nc.tensor.matmul(
                                avp[:, b2, t, :],
                                vd[32 * b:32 * b + 32, 32 * B + t, :],
                                att[32 * b:32 * b + 32, 16 * t:16 * t + 16],
                                start=True,
                                stop=True,
                                tile_position=(32 * b, 0),
                            )
                    # avp[64w+d, b2, 2m+rho, q] -> oft2[64(q%2)+d, q//2,
                    #   256B + 128rho + 64w + 32bh + 16b2 + m]
                    srcx = avp[:].rearrange(
                        "p b (m r) (g s) -> p g r (b m) s", r=2, s=2)
                    for w in range(2):
                        for sg in range(2):
                            srcc = srcx[64 * w:64 * w + 64, :, :, :, sg]
                            dst = dstx[64 * sg:64 * sg + 64, :, :, B,
                                       64 * w + 32 * bh:64 * w + 32 * bh + 32]
                            if (w + sg + B + bh) % 2 == 0:
                                nc.vector.tensor_copy(dst, srcc)
                            else:
                                nc.scalar.copy(dst, srcc)

            def wo_out(p):
                """y^T = Wo^T-chunks @ oft2, DMA out (bias added on host)."""
                for c in range(8):
                    # rotates through the ep buffers (attention is done
                    # with them by now) -> double-buffered Wo psum at no
                    # extra bank cost
                    ytp = pe_pool.tile([128, RC], F32, tag="ep")
                    for g in range(8):
                        nc.tensor.matmul(
                            ytp[:],
                            wo_sb[:, g, 128 * c:128 * c + 128],
                            oft2[:, g, :],
                            start=(g == 0),
                            stop=(g == 7),
                        )
                    ys = ypool.tile([128, RC], F32, tag="ys")
                    if c % 2 == 0:
                        nc.vector.tensor_copy(ys[:], ytp[:])
                    else:
                        nc.scalar.copy(ys[:], ytp[:])
                    nc.scalar.dma_start(
                        yt.rearrange("(t q) r -> q t r", q=128)[
                            :, c, p * RC:(p + 1) * RC
                        ],
                        ys[:],
                    )

            xtcs = {0: xtc0}
            for it in range(NP + 1):
                do_proj = it < NP
                do_att = it >= 1
                p = it - 1
                if do_proj:
                    xtc = xtcs.pop(it)
                    # prefetch next pass's x chunk a full iteration ahead
                    if it + 1 < NP:
                        r0 = (it + 1) * RC
                        xn = xpool.tile([128, 8, RC], BF16, tag="xtc")
                        for c in range(8):
                            nc.sync.dma_start(
                                xn[:, c, :], xtr[:, c, r0:r0 + RC])
                        xtcs[it + 1] = xn

                # --- interleaved tensor-queue schedule ---
                if do_proj:
                    q_stg = proj_tr("q", wq_sb, xtc)
                    relayout_q(it, q_stg)
                if do_att:
                    att0 = energy_softmax(0, p % 2)
                if do_proj:
                    k_stg = proj_tr("k", wk_sb, xtc)
                    relayout_k(it, k_stg)
                if do_att:
                    att1 = energy_softmax(1, p % 2)
                    av_extract(0, att0, p % 2)
                if do_proj:
                    proj_v(it, xtc)
                if do_att:
                    av_extract(1, att1, p % 2)
                    wo_out(p)

    nc.finalize()
    return nc


_CACHE = {}


def _get_nc(R, RC):
    key = (R, RC)
    if key not in _CACHE:
        _CACHE[key] = build_nc(R, RC)
    return _CACHE[key]


def run_cores(x2d, Wq, Wk, Wv, Wo, bo_v, R=None, RC=512, cores=None,
              **run_kwargs):
    """x2d: (ROWS, E) fp32.  Returns (ROWS, E) fp32."""
    ROWS = x2d.shape[0]
    if cores is None:
        cores = list(range(NCORE))
    n = len(cores)
    if R is None:
        R = ROWS // n
    assert R * n == ROWS
    nc = _get_nc(R, RC)

    bf = ml_dtypes.bfloat16
    scale = 1.0 / np.sqrt(np.sqrt(float(E)))  # fold E**-0.5 into both Wq, Wk
    wq_b = (Wq.astype(np.float64) * scale).astype(bf)
    wk_b = (Wk.astype(np.float64) * scale).astype(bf)
    wv_b = Wv.astype(bf)
    wo_b = Wo.astype(bf)
    bo_f = bo_v.reshape(1, E).astype(np.float32)

    in_maps = []
    for ci in range(n):
        xs = x2d[ci * R:(ci + 1) * R].T  # (E, R)
        in_maps.append({
            "xt": np.ascontiguousarray(xs).astype(bf),
            "wq": wq_b, "wk": wk_b, "wv": wv_b, "wo": wo_b,
        })
    res = run_bass_kernel_spmd(nc, in_maps, core_ids=cores, **run_kwargs)
    out = np.empty((ROWS, E), dtype=np.float32)
    for ci in range(n):
        ytd = res.results[ci]["yt"]  # (E, R), columns in natural row order
        out[ci * R:(ci + 1) * R] = ytd.T + bo_f  # bias added on host
    if run_kwargs.get("trace"):
        return out, res
    return out


def kernel(x, Wq, Wk, Wv, Wo, bo):
    x = np.asarray(x, dtype=np.float32)
    N, L, _ = x.shape
    y = run_cores(
        x.reshape(N * L, E),
        np.asarray(Wq, np.float32), np.asarray(Wk, np.float32),
        np.asarray(Wv, np.float32), np.asarray(Wo, np.float32),
        np.asarray(bo, np.float32),
    )
    return y.reshape(N, L, E)


# revision 13
# speedup vs baseline: 1.0136x; 1.0136x over previous
"""Trainium2 Bass kernel for the cross-head MultiHeadAttention module.

Reference computation (per row r of x flattened to (N*L, E)):
    q = x @ Wq; k = x @ Wk; v = x @ Wv           (E = 1024, H = 16, D = 64)
    energy[r, i, j] = sum_d q[r,i,d] * k[r,j,d]  (cross-head, per position)
    attn = softmax(energy / 32, axis=j)
    out[r, i, :] = sum_j attn[r,i,j] * v[r,j,:]
    y = out.reshape(R, E) @ Wo + bo

Distribution: data-parallel over rows (N*L = 16384 -> 2048 rows/core x 8).

Per-core design (all big matmuls in bf16 on the PE array), v5:
  *  Q/K projections run transposed (features on partitions, rows free).
     Their attention-layout rebuild (qd2b/kht2 block-diagonal slabs) is a
     pure partition-shift, so it runs as 32 direct SBUF->SBUF DMAs per
     tensor (512B runs both sides) with NO DRAM round trip; each DMA only
     depends on one projection chunk's psum->sbuf copy, so the rebuild
     streams behind the projection instead of after it.
  *  V runs natural (rows on partitions) and still round-trips through
     DRAM (its rebuild needs a partition<->free exchange which SBUF APs
     cannot express).  vd is double-buffered by pass parity so the
     readback issues immediately after the stage-out instead of waiting
     for the previous pass's AV matmuls (WAR removed).
  *  Energy: ONE matmul per row pair (pi, pi+RC/2): lhsT = qd2b[:, :, pi]
     ([128, 32] block-diagonal), rhs = kht2[:, :, pi] ([128, 16]); out is
     a dense [32, 16] block of a 256-row psum bank so softmax runs on
     dense [128, 512] tiles.
  *  softmax: exp (no max-subtraction: energies ~N(0, 1/16)), row-sum,
     reciprocal, scale+cast-to-bf16, 32x32-block vector transpose.
  *  A@V: ONE matmul per row pair; avp psum is 2 double-buffered 2-bank
     tiles (b-halves) so bank1's matmuls only wait on bank0's first-half
     extraction.
  *  y^T: full-width Wo matmuls accumulated in psum; + bo; DMA out.
  *  Schedule: the tensor queue interleaves pass p-1's attention with
     pass p's projections at chunk granularity:
       projq(p) | energy0(p-1) | projk(p) | energy1(p-1) | av0(p-1) |
       projv(p)+vd-readback(p) | av1(p-1) | Wo(p-1)
     so softmax / extraction / readback chains hide under projection
     matmuls and the PE p-state stays high.  Startup DMAs are split
     per-128-column chunk and ordered xtc -> Wq -> Wk -> Wv -> Wo.
"""

import numpy as np
import ml_dtypes

import concourse.bass as bass
from concourse import bacc
import concourse.tile as tile
from concourse import mybir
from concourse.bass_utils import run_bass_kernel_spmd

F32 = mybir.dt.float32
BF16 = mybir.dt.bfloat16
AF = mybir.ActivationFunctionType
ALU = mybir.AluOpType
AX = mybir.AxisListType

E = 1024
H = 16
D = 64
NCORE = 8


def build_nc(R, RC):
    """Per-core kernel program: R rows total, processed in passes of RC."""
    NP = R // RC          # passes
    NBK = RC // 256       # dense energy banks per pass (256 rows each)
    PH = RC // 2          # row pairs per pass

    nc = bacc.Bacc("TRN2", target_bir_lowering=False, debug=False)

    xt = nc.dram_tensor("xt", [E, R], BF16, kind="ExternalInput")
    wq = nc.dram_tensor("wq", [E, E], BF16, kind="ExternalInput")
    wk = nc.dram_tensor("wk", [E, E], BF16, kind="ExternalInput")
    wv = nc.dram_tensor("wv", [E, E], BF16, kind="ExternalInput")
    wo = nc.dram_tensor("wo", [E, E], BF16, kind="ExternalInput")
    yt = nc.dram_tensor("yt", [E, R], F32, kind="ExternalOutput")

    with tile.TileContext(nc) as tc:
        with (
            tc.tile_pool(name="wpool", bufs=1) as wpool,      # persistent
            tc.tile_pool(name="xpool", bufs=2) as xpool,      # xt chunks
            tc.tile_pool(name="spool", bufs=1) as spool,      # q/k/v staging
            tc.tile_pool(name="apool", bufs=2) as apool,      # softmax temps
            tc.tile_pool(name="opool", bufs=1) as opool,      # oft2
            tc.tile_pool(name="ypool", bufs=2) as ypool,      # y staging
            tc.tile_pool(name="dram", bufs=2, space="DRAM") as dpool,
            tc.tile_pool(name="pproj", bufs=2, space="PSUM") as pproj,
            tc.tile_pool(name="pe", bufs=2, space="PSUM") as pe_pool,
            tc.tile_pool(name="pav", bufs=2, space="PSUM") as pav,
        ):
            # ---- persistent loads (split per 128-col chunk; xtc(0) + wq
            # first so the first projection starts as early as possible).
            # DMA traffic classes: latency-critical relayouts (qk, vd) +
            # xtc go on the SP HWDGE (nc.sync); bulk streams (weights,
            # V stage-out, y out) go on the Activation HWDGE (nc.scalar)
            # so bulk descriptor storms never sit ahead of critical ones.
            wq_sb = wpool.tile([128, 8, E], BF16, tag="wq")
            wk_sb = wpool.tile([128, 8, E], BF16, tag="wk")
            wv_sb = wpool.tile([128, 8, E], BF16, tag="wv")
            wo_sb = wpool.tile([128, 8, E], BF16, tag="wo")

            xtr = xt.rearrange("(c p) r -> p c r", p=128)
            xtc0 = xpool.tile([128, 8, RC], BF16, tag="xtc")
            for c in range(8):
                nc.sync.dma_start(xtc0[:, c, :], xtr[:, c, 0:RC])
            for w_sb, w_d in ((wq_sb, wq), (wk_sb, wk), (wv_sb, wv),
                              (wo_sb, wo)):
                wr = w_d.rearrange("(c p) e -> p c e", p=128)
                for c in range(8):
                    nc.sync.dma_start(w_sb[:, c, :], wr[:, c, :])

            # block-diagonal operand tiles: zero blocks are memset once and
            # never rewritten (per-pass DMAs touch only the data blocks).
            # qd2b/kht2/vd ping-pong per pass parity so pass p+1's rebuild
            # DMAs never wait on pass p's attention matmuls.
            qd2bs, kht2s, vds = [], [], []
            for pp in range(2):
                qd2b_ = wpool.tile([128, 32, PH], BF16, tag=f"qd2b{pp}",
                                   name=f"qd2b{pp}")
                nc.vector.memset(qd2b_[0:64, 16:32, :], 0.0)
                nc.vector.memset(qd2b_[64:128, 0:16, :], 0.0)
                qd2bs.append(qd2b_)
                kht2_ = wpool.tile([128, 16, PH], BF16, tag=f"kht2{pp}",
                                   name=f"kht2{pp}")
                kht2s.append(kht2_)
                vd_ = wpool.tile([128, NBK * 32, 128], BF16, tag=f"vd{pp}",
                                 name=f"vd{pp}")
                nc.vector.memset(vd_[:], 0.0)
                vds.append(vd_)

            oft2 = opool.tile([128, 8, RC], BF16, tag="oft2")

            def proj_tr(name, w_sb, xtc):
                """Transposed projection (features on partitions)."""
                stg = spool.tile([128, 8, RC], BF16, tag=f"stg_{name}")
                for et in range(8):
                    ps = pproj.tile([128, RC], F32, tag="proj")
                    for c in range(8):
                        nc.tensor.matmul(
                            ps[:],
                            w_sb[:, c, et * 128:(et + 1) * 128],
                            xtc[:, c, :],
                            start=(c == 0),
                            stop=(c == 7),
                        )
                    if et % 2 == 0:
                        nc.vector.tensor_copy(stg[:, et, :], ps[:])
                    else:
                        nc.scalar.copy(stg[:, et, :], ps[:])
                return stg

            def relayout_q(it, stg):
                """stg_q -> qd2b[it%2] via 32 partition-shift SBUF DMAs.

                qd2b[64h'+d, 16h'+q, pi] = q^T[d, head q, row 256h'+pi]
                and stg[64(q%2)+d, q//2, r] = q^T[d, head q, row r].
                """
                qd2b_w = qd2bs[it % 2]
                for hp in range(2):
                    for q in range(H):
                        nc.sync.dma_start(
                            qd2b_w[64 * hp:64 * hp + 64, 16 * hp + q, :],
                            stg[64 * (q % 2):64 * (q % 2) + 64, q // 2,
                                256 * hp:256 * hp + 256],
                        )

            def relayout_k(it, stg):
                """stg_k -> kht2[it%2]: kht2[64h'+d, q, pi] = k^T[d, q,
                row 256h'+pi]."""
                kht2_w = kht2s[it % 2]
                for hp in range(2):
                    for q in range(H):
                        nc.sync.dma_start(
                            kht2_w[64 * hp:64 * hp + 64, q, :],
                            stg[64 * (q % 2):64 * (q % 2) + 64, q // 2,
                                256 * hp:256 * hp + 256],
                        )

            def proj_v(it, xtc):
                """Natural (row-major) V projection, staged to DRAM per
                128-row chunk (chunk rc_ = rows with (h, B) =
                divmod(rc_, NBK))."""
                vstg = spool.tile([128, RC // 128, E], BF16, tag="stg_v")
                v2d = dpool.tile([RC, E], BF16, tag="dram_v")
                v2dr = v2d[:].rearrange("(rc p) e -> p rc e", p=128)
                for rc_ in range(RC // 128):
                    for h2 in range(2):
                        ps = pproj.tile([128, 512], F32, tag="proj")
                        for c in range(8):
                            nc.tensor.matmul(
                                ps[:],
                                xtc[:, c, rc_ * 128:(rc_ + 1) * 128],
                                wv_sb[:, c, h2 * 512:(h2 + 1) * 512],
                                start=(c == 0),
                                stop=(c == 7),
                            )
                        if h2 == 0:
                            nc.vector.tensor_copy(vstg[:, rc_, 0:512], ps[:])
                        else:
                            nc.scalar.copy(vstg[:, rc_, 512:1024], ps[:])
                    nc.sync.dma_start(v2dr[:, rc_, :], vstg[:, rc_, :])
                return v2d

            def vd_readback(it, v2d):
                """DRAM -> vd[it%2] block-diagonal V slabs."""
                vd_w = vds[it % 2]
                vsrc = v2d[:].rearrange(
                    "(h B w b m) (j d) -> h w B b j m d",
                    h=2, B=NBK, w=2, b=4, m=16, j=16,
                )
                for rho in range(2):
                    for B in range(NBK):
                        for w in range(2):
                            for b in range(4):
                                nc.sync.dma_start(
                                    vd_w[32 * b + 16 * w:
                                         32 * b + 16 * w + 16,
                                         32 * B + rho:32 * B + 32:2,
                                         64 * w:64 * w + 64],
                                    vsrc[rho, w, B, b],
                                )

            def energy_softmax(B, par):
                """One dense 256-row energy bank + its softmax; returns att."""
                qd2b, kht2 = qd2bs[par], kht2s[par]
                ep = pe_pool.tile([128, 32, 16], F32, tag="ep")
                for lam in range(128):
                    pi = 128 * B + lam
                    b, s = (lam // 16) % 4, 2 * (lam % 16) + lam // 64
                    nc.tensor.matmul(
                        ep[32 * b:32 * b + 32, s, :],
                        qd2b[:, :, pi],
                        kht2[:, :, pi],
                        start=True,
                        stop=True,
                        tile_position=(0, 32 * b),
                    )
                ex = apool.tile([128, 32, 16], F32, tag="ex")
                nc.scalar.activation(ex[:], ep[:], AF.Exp)
                sm = apool.tile([128, 32], F32, tag="sm")
                nc.vector.reduce_sum(sm[:], ex[:], axis=AX.X)
                rcp = apool.tile([128, 32], F32, tag="rcp")
                nc.vector.reciprocal(rcp[:], sm[:])
                at = apool.tile([128, 32, 16], BF16, tag="at")
                nc.vector.tensor_tensor(
                    at[:], ex[:],
                    rcp[:, :, None].to_broadcast([128, 32, 16]),
                    ALU.mult,
                )
                att = apool.tile([128, 512], BF16, tag="att")
                nc.vector.transpose(att[:], at[:].rearrange("p a b -> p (a b)"))
                return att

            def av_extract(B, att, par):
                """A@V for bank B: two 2-bank psum tiles (b-halves) + 8
                merged extraction copies."""
                vd = vds[par]
                dstx = oft2[:].rearrange(
                    "p g (h Bk wc) -> p g h Bk wc", h=2, Bk=NBK)
                for bh in range(2):
                    avp = pav.tile([128, 2, 32, 16], F32, tag="avp")
                    for b2 in range(2):
                        b = 2 * bh + b2
                        for t in range(32):
                            nc.tensor.matmul(
                                avp[:, b2, t, :],
                                vd[32 * b:32 * b + 32, 32 * B + t, :],
                                att[32 * b:32 * b + 32, 16 * t:16 * t + 16],
                                start=True,
                                stop=True,
                                tile_position=(32 * b, 0),
                            )
                    # avp[64w+d, b2, 2m+rho, q] -> oft2[64(q%2)+d, q//2,
                    #   256B + 128rho + 64w + 32bh + 16b2 + m]
                    srcx = avp[:].rearrange(
                        "p b (m r) (g s) -> p g r (b m) s", r=2, s=2)
                    for w in range(2):
                        for sg in range(2):
                            srcc = srcx[64 * w:64 * w + 64, :, :, :, sg]
                            dst = dstx[64 * sg:64 * sg + 64, :, :, B,
                                       64 * w + 32 * bh:64 * w + 32 * bh + 32]
                            if (w + sg + B + bh) % 2 == 0:
                                nc.vector.tensor_copy(dst, srcc)
                            else:
                                nc.scalar.copy(dst, srcc)

            def wo_out(p):
                """y^T = Wo^T-chunks @ oft2, DMA out (bias added on host)."""
                for c in range(8):
                    # rotates through the ep buffers (attention is done
                    # with them by now) -> double-buffered Wo psum at no
                    # extra bank cost
                    ytp = pe_pool.tile([128, RC], F32, tag="ep")
                    for g in range(8):
                        nc.tensor.matmul(
                            ytp[:],
                            wo_sb[:, g, 128 * c:128 * c + 128],
                            oft2[:, g, :],
                            start=(g == 0),
                            stop=(g == 7),
                        )
                    ys = ypool.tile([128, RC], F32, tag="ys")
                    if c % 2 == 0:
                        nc.vector.tensor_copy(ys[:], ytp[:])
                    else:
                        nc.scalar.copy(ys[:], ytp[:])
                    nc.scalar.dma_start(
                        yt.rearrange("(t q) r -> q t r", q=128)[
                            :, c, p * RC:(p + 1) * RC
                        ],
                        ys[:],
                    )

            # Depth-3 software pipeline: attention for pass p runs during
            # iteration p+2, so every relayout DMA has ~1.5 iterations of
            # slack before its consumer — DMA-ring jitter and sim-vs-HW
            # timing divergence can no longer stall the tensor pipe.
            # Buffer parity it%2 serves passes it, it+2, ...; the WAR
            # ordering (attention(p) reads before relayout(p+2) overwrites
            # the same-parity buffer) is enforced by issuing the energy/AV
            # blocks before the same iteration's relayouts.
            xtcs = {0: xtc0}
            for it in range(NP + 2):
                do_proj = it < NP
                do_att = it >= 2
                p = it - 2
                if do_proj:
                    xtc = xtcs.pop(it)
                    # prefetch next pass's x chunk a full iteration ahead
                    if it + 1 < NP:
                        r0 = (it + 1) * RC
                        xn = xpool.tile([128, 8, RC], BF16, tag="xtc")
                        for c in range(8):
                            nc.sync.dma_start(
                                xn[:, c, :], xtr[:, c, r0:r0 + RC])
                        xtcs[it + 1] = xn

                # --- interleaved tensor-queue schedule ---
                if do_proj:
                    q_stg = proj_tr("q", wq_sb, xtc)
                if do_att:
                    att0 = energy_softmax(0, p % 2)
                if do_proj:
                    k_stg = proj_tr("k", wk_sb, xtc)
                if do_att:
                    att1 = energy_softmax(1, p % 2)
                if do_proj:
                    # both energy banks have read qd2b/kht2[it%2] by here
                    relayout_q(it, q_stg)
                    relayout_k(it, k_stg)
                if do_att:
                    av_extract(0, att0, p % 2)
                if do_proj:
                    v2d = proj_v(it, xtc)
                if do_att:
                    av_extract(1, att1, p % 2)
                if do_proj:
                    # vd[it%2] free once both AV banks of pass it-2 are done
                    vd_readback(it, v2d)
                if do_att:
                    wo_out(p)

    nc.finalize()
    return nc


_CACHE = {}


def _get_nc(R, RC):
    key = (R, RC)
    if key not in _CACHE:
        _CACHE[key] = build_nc(R, RC)
    return _CACHE[key]


def run_cores(x2d, Wq, Wk, Wv, Wo, bo_v, R=None, RC=512, cores=None,
              **run_kwargs):
    """x2d: (ROWS, E) fp32.  Returns (ROWS, E) fp32."""
    ROWS = x2d.shape[0]
    if cores is None:
        cores = list(range(NCORE))
    n = len(cores)
    if R is None:
        R = ROWS // n
    assert R * n == ROWS
    nc = _get_nc(R, RC)

    bf = ml_dtypes.bfloat16
    scale = 1.0 / np.sqrt(np.sqrt(float(E)))  # fold E**-0.5 into both Wq, Wk
    wq_b = (Wq.astype(np.float64) * scale).astype(bf)
    wk_b = (Wk.astype(np.float64) * scale).astype(bf)
    wv_b = Wv.astype(bf)
    wo_b = Wo.astype(bf)
    bo_f = bo_v.reshape(1, E).astype(np.float32)

    in_maps = []
    for ci in range(n):
        xs = x2d[ci * R:(ci + 1) * R].T  # (E, R)
        in_maps.append({
            "xt": np.ascontiguousarray(xs).astype(bf),
            "wq": wq_b, "wk": wk_b, "wv": wv_b, "wo": wo_b,
        })
    res = run_bass_kernel_spmd(nc, in_maps, core_ids=cores, **run_kwargs)
    out = np.empty((ROWS, E), dtype=np.float32)
    for ci in range(n):
        ytd = res.results[ci]["yt"]  # (E, R), columns in natural row order
        out[ci * R:(ci + 1) * R] = ytd.T + bo_f  # bias added on host
    if run_kwargs.get("trace"):
        return out, res
    return out


def kernel(x, Wq, Wk, Wv, Wo, bo):
    x = np.asarray(x, dtype=np.float32)
    N, L, _ = x.shape
    y = run_cores(
        x.reshape(N * L, E),
        np.asarray(Wq, np.float32), np.asarray(Wk, np.float32),
        np.asarray(Wv, np.float32), np.asarray(Wo, np.float32),
        np.asarray(bo, np.float32),
    )
    return y.reshape(N, L, E)


# revision 22
# speedup vs baseline: 1.0191x; 1.0054x over previous
"""Trainium2 Bass kernel for the cross-head MultiHeadAttention module.

Reference computation (per row r of x flattened to (N*L, E)):
    q = x @ Wq; k = x @ Wk; v = x @ Wv           (E = 1024, H = 16, D = 64)
    energy[r, i, j] = sum_d q[r,i,d] * k[r,j,d]  (cross-head, per position)
    attn = softmax(energy / 32, axis=j)
    out[r, i, :] = sum_j attn[r,i,j] * v[r,j,:]
    y = out.reshape(R, E) @ Wo + bo

Distribution: data-parallel over rows (N*L = 16384 -> 2048 rows/core x 8).

Per-core design (all big matmuls in bf16 on the PE array), v5:
  *  Q/K projections run transposed (features on partitions, rows free).
     Their attention-layout rebuild (qd2b/kht2 block-diagonal slabs) is a
     pure partition-shift, so it runs as 32 direct SBUF->SBUF DMAs per
     tensor (512B runs both sides) with NO DRAM round trip; each DMA only
     depends on one projection chunk's psum->sbuf copy, so the rebuild
     streams behind the projection instead of after it.
  *  V runs natural (rows on partitions) and still round-trips through
     DRAM (its rebuild needs a partition<->free exchange which SBUF APs
     cannot express).  vd is double-buffered by pass parity so the
     readback issues immediately after the stage-out instead of waiting
     for the previous pass's AV matmuls (WAR removed).
  *  Energy: ONE matmul per row pair (pi, pi+RC/2): lhsT = qd2b[:, :, pi]
     ([128, 32] block-diagonal), rhs = kht2[:, :, pi] ([128, 16]); out is
     a dense [32, 16] block of a 256-row psum bank so softmax runs on
     dense [128, 512] tiles.
  *  softmax: exp (no max-subtraction: energies ~N(0, 1/16)), row-sum,
     reciprocal, scale+cast-to-bf16, 32x32-block vector transpose.
  *  A@V: ONE matmul per row pair; avp psum is 2 double-buffered 2-bank
     tiles (b-halves) so bank1's matmuls only wait on bank0's first-half
     extraction.
  *  y^T: full-width Wo matmuls accumulated in psum; + bo; DMA out.
  *  Schedule: the tensor queue interleaves pass p-1's attention with
     pass p's projections at chunk granularity:
       projq(p) | energy0(p-1) | projk(p) | energy1(p-1) | av0(p-1) |
       projv(p)+vd-readback(p) | av1(p-1) | Wo(p-1)
     so softmax / extraction / readback chains hide under projection
     matmuls and the PE p-state stays high.  Startup DMAs are split
     per-128-column chunk and ordered xtc -> Wq -> Wk -> Wv -> Wo.
"""

import numpy as np
import ml_dtypes

import concourse.bass as bass
from concourse import bacc
import concourse.tile as tile
from concourse import mybir
from concourse.bass_utils import run_bass_kernel_spmd

F32 = mybir.dt.float32
BF16 = mybir.dt.bfloat16
AF = mybir.ActivationFunctionType
ALU = mybir.AluOpType
AX = mybir.AxisListType

E = 1024
H = 16
D = 64
NCORE = 8


def build_nc(R, RC):
    """Per-core kernel program: R rows total, processed in passes of RC."""
    NP = R // RC          # passes
    NBK = RC // 256       # dense energy banks per pass (256 rows each)
    PH = RC // 2          # row pairs per pass

    nc = bacc.Bacc("TRN2", target_bir_lowering=False, debug=False)

    xt = nc.dram_tensor("xt", [E, R], BF16, kind="ExternalInput")
    wq = nc.dram_tensor("wq", [E, E], BF16, kind="ExternalInput")
    wk = nc.dram_tensor("wk", [E, E], BF16, kind="ExternalInput")
    wv = nc.dram_tensor("wv", [E, E], BF16, kind="ExternalInput")
    wo = nc.dram_tensor("wo", [E, E], BF16, kind="ExternalInput")
    yt = nc.dram_tensor("yt", [E, R], F32, kind="ExternalOutput")

    with tile.TileContext(nc) as tc:
        with (
            tc.tile_pool(name="wpool", bufs=1) as wpool,      # persistent
            tc.tile_pool(name="xpool", bufs=2) as xpool,      # xt chunks
            tc.tile_pool(name="spool", bufs=1) as spool,      # q/k/v staging
            tc.tile_pool(name="apool", bufs=2) as apool,      # softmax temps
            tc.tile_pool(name="opool", bufs=1) as opool,      # oft2
            tc.tile_pool(name="ypool", bufs=2) as ypool,      # y staging
            tc.tile_pool(name="dram", bufs=2, space="DRAM") as dpool,
            tc.tile_pool(name="pproj", bufs=2, space="PSUM") as pproj,
            tc.tile_pool(name="pe", bufs=2, space="PSUM") as pe_pool,
            tc.tile_pool(name="pav", bufs=2, space="PSUM") as pav,
        ):
            # ---- persistent loads (split per 128-col chunk; xtc(0) + wq
            # first so the first projection starts as early as possible).
            # DMA traffic classes: latency-critical relayouts (qk, vd) +
            # xtc go on the SP HWDGE (nc.sync); bulk streams (weights,
            # V stage-out, y out) go on the Activation HWDGE (nc.scalar)
            # so bulk descriptor storms never sit ahead of critical ones.
            wq_sb = wpool.tile([128, 8, E], BF16, tag="wq")
            wk_sb = wpool.tile([128, 8, E], BF16, tag="wk")
            wv_sb = wpool.tile([128, 8, E], BF16, tag="wv")
            wo_sb = wpool.tile([128, 8, E], BF16, tag="wo")

            xtr = xt.rearrange("(c p) r -> p c r", p=128)
            xtc0 = xpool.tile([128, 8, RC], BF16, tag="xtc")
            nc.sync.dma_start(xtc0[:], xtr[:, :, 0:RC])
            for w_sb, w_d in ((wq_sb, wq), (wk_sb, wk), (wv_sb, wv),
                              (wo_sb, wo)):
                nc.sync.dma_start(
                    w_sb[:], w_d.rearrange("(c p) e -> p c e", p=128))

            # block-diagonal operand tiles: zero blocks are memset once and
            # never rewritten (per-pass DMAs touch only the data blocks).
            # qd2b/kht2/vd ping-pong per pass parity so pass p+1's rebuild
            # DMAs never wait on pass p's attention matmuls.
            qd2bs, kht2s, vds = [], [], []
            for pp in range(2):
                qd2b_ = wpool.tile([128, 32, PH], BF16, tag=f"qd2b{pp}",
                                   name=f"qd2b{pp}")
                nc.vector.memset(qd2b_[0:64, 16:32, :], 0.0)
                nc.vector.memset(qd2b_[64:128, 0:16, :], 0.0)
                qd2bs.append(qd2b_)
                kht2_ = wpool.tile([128, 16, PH], BF16, tag=f"kht2{pp}",
                                   name=f"kht2{pp}")
                kht2s.append(kht2_)
                vd_ = wpool.tile([128, NBK * 32, 128], BF16, tag=f"vd{pp}",
                                 name=f"vd{pp}")
                nc.vector.memset(vd_[:], 0.0)
                vds.append(vd_)

            oft2 = opool.tile([128, 8, RC], BF16, tag="oft2")

            def proj_tr(name, w_sb, xtc):
                """Transposed projection (features on partitions)."""
                stg = spool.tile([128, 8, RC], BF16, tag=f"stg_{name}")
                for et in range(8):
                    ps = pproj.tile([128, RC], F32, tag="proj")
                    for c in range(8):
                        nc.tensor.matmul(
                            ps[:],
                            w_sb[:, c, et * 128:(et + 1) * 128],
                            xtc[:, c, :],
                            start=(c == 0),
                            stop=(c == 7),
                        )
                    if et % 2 == 0:
                        nc.vector.tensor_copy(stg[:, et, :], ps[:])
                    else:
                        nc.scalar.copy(stg[:, et, :], ps[:])
                return stg

            def relayout_q(it, stg):
                """stg_q -> qd2b[it%2] via 4 partition-shift SBUF DMAs.

                qd2b[64h'+d, 16h'+q, pi] = q^T[d, head q, row 256h'+pi]
                and stg[64(q%2)+d, q//2, r] = q^T[d, head q, row r], so
                each (h', q%2) group of 8 heads is one strided DMA.
                """
                qd2b_w = qd2bs[it % 2]
                for hp in range(2):
                    for q in range(H):
                        nc.sync.dma_start(
                            qd2b_w[64 * hp:64 * hp + 64, 16 * hp + q, :],
                            stg[64 * (q % 2):64 * (q % 2) + 64, q // 2,
                                256 * hp:256 * hp + 256],
                        )

            def relayout_k(it, stg):
                """stg_k -> kht2[it%2]: kht2[64h'+d, q, pi] = k^T[d, q,
                row 256h'+pi]."""
                kht2_w = kht2s[it % 2]
                for hp in range(2):
                    for q in range(H):
                        nc.sync.dma_start(
                            kht2_w[64 * hp:64 * hp + 64, q, :],
                            stg[64 * (q % 2):64 * (q % 2) + 64, q // 2,
                                256 * hp:256 * hp + 256],
                        )

            def proj_v(it, xtc):
                """Natural (row-major) V projection, staged to DRAM per
                128-row chunk."""
                vstg = spool.tile([128, RC // 128, E], BF16, tag="stg_v")
                v2d = dpool.tile([RC, E], BF16, tag="dram_v")
                v2dr = v2d[:].rearrange("(rc p) e -> p rc e", p=128)
                for rc_ in range(RC // 128):
                    for h2 in range(2):
                        ps = pproj.tile([128, 512], F32, tag="proj")
                        for c in range(8):
                            nc.tensor.matmul(
                                ps[:],
                                xtc[:, c, rc_ * 128:(rc_ + 1) * 128],
                                wv_sb[:, c, h2 * 512:(h2 + 1) * 512],
                                start=(c == 0),
                                stop=(c == 7),
                            )
                        if h2 == 0:
                            nc.vector.tensor_copy(vstg[:, rc_, 0:512], ps[:])
                        else:
                            nc.scalar.copy(vstg[:, rc_, 512:1024], ps[:])
                    nc.scalar.dma_start(v2dr[:, rc_, :], vstg[:, rc_, :])
                return v2d

            def vd_readback(it, v2d):
                """DRAM -> vd[it%2] block-diagonal V slabs."""
                vd_w = vds[it % 2]
                vsrc = v2d[:].rearrange(
                    "(h B w b m) (j d) -> h w B b j m d",
                    h=2, B=NBK, w=2, b=4, m=16, j=16,
                )
                for rho in range(2):
                    for B in range(NBK):
                        for w in range(2):
                            for b in range(4):
                                nc.sync.dma_start(
                                    vd_w[32 * b + 16 * w:
                                         32 * b + 16 * w + 16,
                                         32 * B + rho:32 * B + 32:2,
                                         64 * w:64 * w + 64],
                                    vsrc[rho, w, B, b],
                                )

            def energy_softmax(B, par):
                """One dense 256-row energy bank + its softmax; returns att."""
                qd2b, kht2 = qd2bs[par], kht2s[par]
                ep = pe_pool.tile([128, 32, 16], F32, tag="ep")
                for lam in range(128):
                    pi = 128 * B + lam
                    b, s = (lam // 16) % 4, 2 * (lam % 16) + lam // 64
                    nc.tensor.matmul(
                        ep[32 * b:32 * b + 32, s, :],
                        qd2b[:, :, pi],
                        kht2[:, :, pi],
                        start=True,
                        stop=True,
                        tile_position=(0, 32 * b),
                    )
                ex = apool.tile([128, 32, 16], F32, tag="ex")
                nc.scalar.activation(ex[:], ep[:], AF.Exp)
                sm = apool.tile([128, 32], F32, tag="sm")
                nc.vector.reduce_sum(sm[:], ex[:], axis=AX.X)
                rcp = apool.tile([128, 32], F32, tag="rcp")
                nc.vector.reciprocal(rcp[:], sm[:])
                at = apool.tile([128, 32, 16], BF16, tag="at")
                nc.vector.tensor_tensor(
                    at[:], ex[:],
                    rcp[:, :, None].to_broadcast([128, 32, 16]),
                    ALU.mult,
                )
                att = apool.tile([128, 512], BF16, tag="att")
                nc.vector.transpose(att[:], at[:].rearrange("p a b -> p (a b)"))
                return att

            def av_extract(B, att, par):
                """A@V for bank B: two 2-bank psum tiles (b-halves) + 8
                merged extraction copies."""
                vd = vds[par]
                dstx = oft2[:].rearrange(
                    "p g (h Bk wc) -> p g h Bk wc", h=2, Bk=NBK)
                for bh in range(2):
                    avp = pav.tile([128, 2, 32, 16], F32, tag="avp")
                    for b2 in range(2):
                        b = 2 * bh + b2
                        for t in range(32):
                            nc.tensor.matmul(
                                avp[:, b2, t, :],
                                vd[32 * b:32 * b + 32, 32 * B + t, :],
                                att[32 * b:32 * b + 32, 16 * t:16 * t + 16],
                                start=True,
                                stop=True,
                                tile_position=(32 * b, 0),
                            )
                    # avp[64w+d, b2, 2m+rho, q] -> oft2[64(q%2)+d, q//2,
                    #   256B + 128rho + 64w + 32bh + 16b2 + m]
                    srcx = avp[:].rearrange(
                        "p b (m r) (g s) -> p g r (b m) s", r=2, s=2)
                    for w in range(2):
                        for sg in range(2):
                            srcc = srcx[64 * w:64 * w + 64, :, :, :, sg]
                            dst = dstx[64 * sg:64 * sg + 64, :, :, B,
                                       64 * w + 32 * bh:64 * w + 32 * bh + 32]
                            if (w + sg + B + bh) % 2 == 0:
                                nc.vector.tensor_copy(dst, srcc)
                            else:
                                nc.scalar.copy(dst, srcc)

            def wo_out(p):
                """y^T = Wo^T-chunks @ oft2, DMA out (bias added on host)."""
                for c in range(8):
                    # rotates through the ep buffers (attention is done
                    # with them by now) -> double-buffered Wo psum at no
                    # extra bank cost
                    ytp = pe_pool.tile([128, RC], F32, tag="ep")
                    for g in range(8):
                        nc.tensor.matmul(
                            ytp[:],
                            wo_sb[:, g, 128 * c:128 * c + 128],
                            oft2[:, g, :],
                            start=(g == 0),
                            stop=(g == 7),
                        )
                    ys = ypool.tile([128, RC], F32, tag="ys")
                    if c % 2 == 0:
                        nc.vector.tensor_copy(ys[:], ytp[:])
                    else:
                        nc.scalar.copy(ys[:], ytp[:])
                    nc.scalar.dma_start(
                        yt.rearrange("(t q) r -> q t r", q=128)[
                            :, c, p * RC:(p + 1) * RC
                        ],
                        ys[:],
                    )

            # Depth-2 software pipeline: attention for pass p runs during
            # iteration p+1, interleaved with pass p+1's projections at
            # chunk granularity on the tensor queue.  All rebuild targets
            # (qd2b/kht2/vd) ping-pong on pass parity, so iteration it's
            # rebuilds (parity it%2) never touch the buffers pass it-1's
            # attention is reading (parity (it-1)%2).
            xtcs = {0: xtc0}
            for it in range(NP + 1):
                do_proj = it < NP
                do_att = it >= 1
                p = it - 1
                if do_proj:
                    xtc = xtcs.pop(it)
                    # prefetch next pass's x chunk a full iteration ahead
                    if it + 1 < NP:
                        r0 = (it + 1) * RC
                        xn = xpool.tile([128, 8, RC], BF16, tag="xtc")
                        nc.sync.dma_start(xn[:], xtr[:, :, r0:r0 + RC])
                        xtcs[it + 1] = xn

                # --- interleaved tensor-queue schedule ---
                if do_proj:
                    q_stg = proj_tr("q", wq_sb, xtc)
                    relayout_q(it, q_stg)
                if do_att:
                    att0 = energy_softmax(0, p % 2)
                if do_proj:
                    k_stg = proj_tr("k", wk_sb, xtc)
                    relayout_k(it, k_stg)
                if do_att:
                    att1 = energy_softmax(1, p % 2)
                    av_extract(0, att0, p % 2)
                if do_proj:
                    v2d = proj_v(it, xtc)
                    vd_readback(it, v2d)
                if do_att:
                    av_extract(1, att1, p % 2)
                    wo_out(p)

    nc.finalize()
    return nc


_CACHE = {}


def _get_nc(R, RC):
    key = (R, RC)
    if key not in _CACHE:
        _CACHE[key] = build_nc(R, RC)
    return _CACHE[key]


def run_cores(x2d, Wq, Wk, Wv, Wo, bo_v, R=None, RC=512, cores=None,
              **run_kwargs):
    """x2d: (ROWS, E) fp32.  Returns (ROWS, E) fp32."""
    ROWS = x2d.shape[0]
    if cores is None:
        cores = list(range(NCORE))
    n = len(cores)
    if R is None:
        R = ROWS // n
    assert R * n == ROWS
    nc = _get_nc(R, RC)

    bf = ml_dtypes.bfloat16
    scale = 1.0 / np.sqrt(np.sqrt(float(E)))  # fold E**-0.5 into both Wq, Wk
    wq_b = (Wq.astype(np.float64) * scale).astype(bf)
    wk_b = (Wk.astype(np.float64) * scale).astype(bf)
    wv_b = Wv.astype(bf)
    wo_b = Wo.astype(bf)
    bo_f = bo_v.reshape(1, E).astype(np.float32)

    in_maps = []
    for ci in range(n):
        xs = x2d[ci * R:(ci + 1) * R].T  # (E, R)
        in_maps.append({
            "xt": np.ascontiguousarray(xs).astype(bf),
            "wq": wq_b, "wk": wk_b, "wv": wv_b, "wo": wo_b,
        })
    res = run_bass_kernel_spmd(nc, in_maps, core_ids=cores, **run_kwargs)
    out = np.empty((ROWS, E), dtype=np.float32)
    for ci in range(n):
        ytd = res.results[ci]["yt"]  # (E, R), columns in natural row order
        out[ci * R:(ci + 1) * R] = ytd.T + bo_f  # bias added on host
    if run_kwargs.get("trace"):
        return out, res
    return out


def kernel(x, Wq, Wk, Wv, Wo, bo):
    x = np.asarray(x, dtype=np.float32)
    N, L, _ = x.shape
    y = run_cores(
        x.reshape(N * L, E),
        np.asarray(Wq, np.float32), np.asarray(Wk, np.float32),
        np.asarray(Wv, np.float32), np.asarray(Wo, np.float32),
        np.asarray(bo, np.float32),
    )
    return y.reshape(N, L, E)


# revision 23
# speedup vs baseline: 1.0449x; 1.0253x over previous
"""Trainium2 Bass kernel for the cross-head MultiHeadAttention module.

Reference computation (per row r of x flattened to (N*L, E)):
    q = x @ Wq; k = x @ Wk; v = x @ Wv           (E = 1024, H = 16, D = 64)
    energy[r, i, j] = sum_d q[r,i,d] * k[r,j,d]  (cross-head, per position)
    attn = softmax(energy / 32, axis=j)
    out[r, i, :] = sum_j attn[r,i,j] * v[r,j,:]
    y = out.reshape(R, E) @ Wo + bo

Distribution: data-parallel over rows (N*L = 16384 -> 2048 rows/core x 8).

Per-core design (all big matmuls in bf16 on the PE array), v5:
  *  Q/K projections run transposed (features on partitions, rows free).
     Their attention-layout rebuild (qd2b/kht2 block-diagonal slabs) is a
     pure partition-shift, so it runs as 32 direct SBUF->SBUF DMAs per
     tensor (512B runs both sides) with NO DRAM round trip; each DMA only
     depends on one projection chunk's psum->sbuf copy, so the rebuild
     streams behind the projection instead of after it.
  *  V runs natural (rows on partitions) and still round-trips through
     DRAM (its rebuild needs a partition<->free exchange which SBUF APs
     cannot express).  vd is double-buffered by pass parity so the
     readback issues immediately after the stage-out instead of waiting
     for the previous pass's AV matmuls (WAR removed).
  *  Energy: ONE matmul per row pair (pi, pi+RC/2): lhsT = qd2b[:, :, pi]
     ([128, 32] block-diagonal), rhs = kht2[:, :, pi] ([128, 16]); out is
     a dense [32, 16] block of a 256-row psum bank so softmax runs on
     dense [128, 512] tiles.
  *  softmax: exp (no max-subtraction: energies ~N(0, 1/16)), row-sum,
     reciprocal, scale+cast-to-bf16, 32x32-block vector transpose.
  *  A@V: ONE matmul per row pair; avp psum is 2 double-buffered 2-bank
     tiles (b-halves) so bank1's matmuls only wait on bank0's first-half
     extraction.
  *  y^T: full-width Wo matmuls accumulated in psum; + bo; DMA out.
  *  Schedule: the tensor queue interleaves pass p-1's attention with
     pass p's projections at chunk granularity:
       projq(p) | energy0(p-1) | projk(p) | energy1(p-1) | av0(p-1) |
       projv(p)+vd-readback(p) | av1(p-1) | Wo(p-1)
     so softmax / extraction / readback chains hide under projection
     matmuls and the PE p-state stays high.  Startup DMAs are split
     per-128-column chunk and ordered xtc -> Wq -> Wk -> Wv -> Wo.
"""

import numpy as np
import ml_dtypes

import concourse.bass as bass
from concourse import bacc
import concourse.tile as tile
from concourse import mybir
from concourse.bass_utils import run_bass_kernel_spmd

F32 = mybir.dt.float32
BF16 = mybir.dt.bfloat16
AF = mybir.ActivationFunctionType
ALU = mybir.AluOpType
AX = mybir.AxisListType

E = 1024
H = 16
D = 64
NCORE = 8


def build_nc(R, RC):
    """Per-core kernel program: R rows total, processed in passes of RC."""
    NP = R // RC          # passes
    NBK = RC // 256       # dense energy banks per pass (256 rows each)
    PH = RC // 2          # row pairs per pass

    nc = bacc.Bacc("TRN2", target_bir_lowering=False, debug=False)

    xt = nc.dram_tensor("xt", [E, R], BF16, kind="ExternalInput")
    wq = nc.dram_tensor("wq", [E, E], BF16, kind="ExternalInput")
    wk = nc.dram_tensor("wk", [E, E], BF16, kind="ExternalInput")
    wv = nc.dram_tensor("wv", [E, E], BF16, kind="ExternalInput")
    wo = nc.dram_tensor("wo", [E, E], BF16, kind="ExternalInput")
    yt = nc.dram_tensor("yt", [E, R], F32, kind="ExternalOutput")

    with tile.TileContext(nc) as tc:
        with (
            tc.tile_pool(name="wpool", bufs=1) as wpool,      # persistent
            tc.tile_pool(name="xpool", bufs=2) as xpool,      # xt chunks
            tc.tile_pool(name="spool", bufs=1) as spool,      # q/k/v staging
            tc.tile_pool(name="apool", bufs=2) as apool,      # softmax temps
            tc.tile_pool(name="opool", bufs=1) as opool,      # oft2
            tc.tile_pool(name="ypool", bufs=2) as ypool,      # y staging
            tc.tile_pool(name="dram", bufs=2, space="DRAM") as dpool,
            tc.tile_pool(name="pproj", bufs=2, space="PSUM") as pproj,
            tc.tile_pool(name="pe", bufs=2, space="PSUM") as pe_pool,
            tc.tile_pool(name="pav", bufs=2, space="PSUM") as pav,
        ):
            # ---- persistent loads (split per 128-col chunk; xtc(0) + wq
            # first so the first projection starts as early as possible).
            # DMA traffic classes: latency-critical relayouts (qk, vd) +
            # xtc go on the SP HWDGE (nc.sync); bulk streams (weights,
            # V stage-out, y out) go on the Activation HWDGE (nc.scalar)
            # so bulk descriptor storms never sit ahead of critical ones.
            wq_sb = wpool.tile([128, 8, E], BF16, tag="wq")
            wk_sb = wpool.tile([128, 8, E], BF16, tag="wk")
            wv_sb = wpool.tile([128, 8, E], BF16, tag="wv")
            wo_sb = wpool.tile([128, 8, E], BF16, tag="wo")

            xtr = xt.rearrange("(c p) r -> p c r", p=128)
            xtc0 = xpool.tile([128, 8, RC], BF16, tag="xtc")
            nc.sync.dma_start(xtc0[:], xtr[:, :, 0:RC])
            for w_sb, w_d in ((wq_sb, wq), (wk_sb, wk), (wv_sb, wv),
                              (wo_sb, wo)):
                nc.sync.dma_start(
                    w_sb[:], w_d.rearrange("(c p) e -> p c e", p=128))

            # block-diagonal operand tiles: zero blocks are memset once and
            # never rewritten (per-pass DMAs touch only the data blocks).
            # qd2b/kht2/vd ping-pong per pass parity so pass p+1's rebuild
            # DMAs never wait on pass p's attention matmuls.
            qd2bs, kht2s, vds = [], [], []
            for pp in range(2):
                qd2b_ = wpool.tile([128, 32, PH], BF16, tag=f"qd2b{pp}",
                                   name=f"qd2b{pp}")
                nc.vector.memset(qd2b_[0:64, 16:32, :], 0.0)
                nc.vector.memset(qd2b_[64:128, 0:16, :], 0.0)
                qd2bs.append(qd2b_)
                kht2_ = wpool.tile([128, 16, PH], BF16, tag=f"kht2{pp}",
                                   name=f"kht2{pp}")
                kht2s.append(kht2_)
                vd_ = wpool.tile([128, NBK * 32, 128], BF16, tag=f"vd{pp}",
                                 name=f"vd{pp}")
                nc.vector.memset(vd_[:], 0.0)
                vds.append(vd_)

            oft2 = opool.tile([128, 8, RC], BF16, tag="oft2")

            def proj_tr(name, w_sb, xtc):
                """Transposed projection (features on partitions)."""
                stg = spool.tile([128, 8, RC], BF16, tag=f"stg_{name}")
                for et in range(8):
                    ps = pproj.tile([128, RC], F32, tag="proj")
                    for c in range(8):
                        nc.tensor.matmul(
                            ps[:],
                            w_sb[:, c, et * 128:(et + 1) * 128],
                            xtc[:, c, :],
                            start=(c == 0),
                            stop=(c == 7),
                        )
                    if et % 2 == 0:
                        nc.vector.tensor_copy(stg[:, et, :], ps[:])
                    else:
                        nc.scalar.copy(stg[:, et, :], ps[:])
                return stg

            def relayout_q(it, stg):
                """stg_q -> qd2b[it%2] via 4 partition-shift SBUF DMAs.

                qd2b[64h'+d, 16h'+q, pi] = q^T[d, head q, row 256h'+pi]
                and stg[64(q%2)+d, q//2, r] = q^T[d, head q, row r], so
                each (h', q%2) group of 8 heads is one strided DMA.
                """
                qd2b_w = qd2bs[it % 2]
                for hp in range(2):
                    for hs in range(2):
                        nc.sync.dma_start(
                            qd2b_w[64 * hp:64 * hp + 64,
                                   16 * hp + hs:16 * hp + 16:2, :],
                            stg[64 * hs:64 * hs + 64, :,
                                256 * hp:256 * hp + 256],
                        )

            def relayout_k(it, stg):
                """stg_k -> kht2[it%2]: kht2[64h'+d, q, pi] = k^T[d, q,
                row 256h'+pi]."""
                kht2_w = kht2s[it % 2]
                for hp in range(2):
                    for hs in range(2):
                        nc.sync.dma_start(
                            kht2_w[64 * hp:64 * hp + 64, hs:16:2, :],
                            stg[64 * hs:64 * hs + 64, :,
                                256 * hp:256 * hp + 256],
                        )

            def proj_v(it, xtc):
                """Natural (row-major) V projection, staged to DRAM with
                rows permuted to (w, b, B, m, h) order so the vd readback
                needs only 8 DMA instructions.  Chunk rc_ holds the rows
                with (h, B) = divmod(rc_, NBK); stage DMAs use plain
                partition slices (16 calls, on the Activation HWDGE)."""
                vstg = spool.tile([128, RC // 128, E], BF16, tag="stg_v")
                v3d = dpool.tile([2, 4, NBK, 16, 2, E], BF16, tag="dram_v")
                for rc_ in range(RC // 128):
                    for h2 in range(2):
                        ps = pproj.tile([128, 512], F32, tag="proj")
                        for c in range(8):
                            nc.tensor.matmul(
                                ps[:],
                                xtc[:, c, rc_ * 128:(rc_ + 1) * 128],
                                wv_sb[:, c, h2 * 512:(h2 + 1) * 512],
                                start=(c == 0),
                                stop=(c == 7),
                            )
                        if h2 == 0:
                            nc.vector.tensor_copy(vstg[:, rc_, 0:512], ps[:])
                        else:
                            nc.scalar.copy(vstg[:, rc_, 512:1024], ps[:])
                    h, B = divmod(rc_, NBK)
                    for w in range(2):
                        for b in range(4):
                            nc.scalar.dma_start(
                                v3d[w, b, B, :, h, :],
                                vstg[64 * w + 16 * b:64 * w + 16 * b + 16,
                                     rc_, :],
                            )
                return v3d

            def vd_readback(it, v3d):
                """DRAM -> vd[it%2] block-diagonal V slabs: one DMA per
                (row-half w2, quarter b), 3-dim APs on both sides."""
                vd_w = vds[it % 2]
                for w2 in range(2):
                    for b in range(4):
                        nc.sync.dma_start(
                            vd_w[32 * b + 16 * w2:32 * b + 16 * w2 + 16,
                                 :, 64 * w2:64 * w2 + 64],
                            v3d[w2, b].rearrange(
                                "B m h (j d) -> j (B m h) d", j=16, d=64),
                        )

            def energy_softmax(B, par):
                """One dense 256-row energy bank + its softmax; returns att."""
                qd2b, kht2 = qd2bs[par], kht2s[par]
                ep = pe_pool.tile([128, 32, 16], F32, tag="ep")
                for lam in range(128):
                    pi = 128 * B + lam
                    b, s = (lam // 16) % 4, 2 * (lam % 16) + lam // 64
                    nc.tensor.matmul(
                        ep[32 * b:32 * b + 32, s, :],
                        qd2b[:, :, pi],
                        kht2[:, :, pi],
                        start=True,
                        stop=True,
                        tile_position=(0, 32 * b),
                    )
                ex = apool.tile([128, 32, 16], F32, tag="ex")
                nc.scalar.activation(ex[:], ep[:], AF.Exp)
                sm = apool.tile([128, 32], F32, tag="sm")
                nc.vector.reduce_sum(sm[:], ex[:], axis=AX.X)
                rcp = apool.tile([128, 32], F32, tag="rcp")
                nc.vector.reciprocal(rcp[:], sm[:])
                at = apool.tile([128, 32, 16], BF16, tag="at")
                nc.vector.tensor_tensor(
                    at[:], ex[:],
                    rcp[:, :, None].to_broadcast([128, 32, 16]),
                    ALU.mult,
                )
                att = apool.tile([128, 512], BF16, tag="att")
                nc.vector.transpose(att[:], at[:].rearrange("p a b -> p (a b)"))
                return att

            def av_extract(B, att, par):
                """A@V for bank B: two 2-bank psum tiles (b-halves) + 8
                merged extraction copies."""
                vd = vds[par]
                dstx = oft2[:].rearrange(
                    "p g (h Bk wc) -> p g h Bk wc", h=2, Bk=NBK)
                for bh in range(2):
                    avp = pav.tile([128, 2, 32, 16], F32, tag="avp")
                    for b2 in range(2):
                        b = 2 * bh + b2
                        for t in range(32):
                            nc.tensor.matmul(
                                avp[:, b2, t, :],
                                vd[32 * b:32 * b + 32, 32 * B + t, :],
                                att[32 * b:32 * b + 32, 16 * t:16 * t + 16],
                                start=True,
                                stop=True,
                                tile_position=(32 * b, 0),
                            )
                    # avp[64w+d, b2, 2m+rho, q] -> oft2[64(q%2)+d, q//2,
                    #   256B + 128rho + 64w + 32bh + 16b2 + m]
                    srcx = avp[:].rearrange(
                        "p b (m r) (g s) -> p g r (b m) s", r=2, s=2)
                    for w in range(2):
                        for sg in range(2):
                            srcc = srcx[64 * w:64 * w + 64, :, :, :, sg]
                            dst = dstx[64 * sg:64 * sg + 64, :, :, B,
                                       64 * w + 32 * bh:64 * w + 32 * bh + 32]
                            if (w + sg + B + bh) % 2 == 0:
                                nc.vector.tensor_copy(dst, srcc)
                            else:
                                nc.scalar.copy(dst, srcc)

            def wo_out(p):
                """y^T = Wo^T-chunks @ oft2, DMA out (bias added on host)."""
                for c in range(8):
                    # rotates through the ep buffers (attention is done
                    # with them by now) -> double-buffered Wo psum at no
                    # extra bank cost
                    ytp = pe_pool.tile([128, RC], F32, tag="ep")
                    for g in range(8):
                        nc.tensor.matmul(
                            ytp[:],
                            wo_sb[:, g, 128 * c:128 * c + 128],
                            oft2[:, g, :],
                            start=(g == 0),
                            stop=(g == 7),
                        )
                    ys = ypool.tile([128, RC], F32, tag="ys")
                    if c % 2 == 0:
                        nc.vector.tensor_copy(ys[:], ytp[:])
                    else:
                        nc.scalar.copy(ys[:], ytp[:])
                    nc.scalar.dma_start(
                        yt.rearrange("(t q) r -> q t r", q=128)[
                            :, c, p * RC:(p + 1) * RC
                        ],
                        ys[:],
                    )

            # Depth-2 software pipeline: attention for pass p runs during
            # iteration p+1, interleaved with pass p+1's projections at
            # chunk granularity on the tensor queue.  All rebuild targets
            # (qd2b/kht2/vd) ping-pong on pass parity, so iteration it's
            # rebuilds (parity it%2) never touch the buffers pass it-1's
            # attention is reading (parity (it-1)%2).
            xtcs = {0: xtc0}
            for it in range(NP + 1):
                do_proj = it < NP
                do_att = it >= 1
                p = it - 1
                if do_proj:
                    xtc = xtcs.pop(it)
                    # prefetch next pass's x chunk a full iteration ahead
                    if it + 1 < NP:
                        r0 = (it + 1) * RC
                        xn = xpool.tile([128, 8, RC], BF16, tag="xtc")
                        nc.sync.dma_start(xn[:], xtr[:, :, r0:r0 + RC])
                        xtcs[it + 1] = xn

                # --- interleaved tensor-queue schedule ---
                if do_proj:
                    q_stg = proj_tr("q", wq_sb, xtc)
                    relayout_q(it, q_stg)
                if do_att:
                    att0 = energy_softmax(0, p % 2)
                if do_proj:
                    k_stg = proj_tr("k", wk_sb, xtc)
                    relayout_k(it, k_stg)
                if do_att:
                    att1 = energy_softmax(1, p % 2)
                    av_extract(0, att0, p % 2)
                if do_proj:
                    v3d = proj_v(it, xtc)
                    vd_readback(it, v3d)
                if do_att:
                    av_extract(1, att1, p % 2)
                    wo_out(p)

    nc.finalize()
    return nc


_CACHE = {}


def _get_nc(R, RC):
    key = (R, RC)
    if key not in _CACHE:
        _CACHE[key] = build_nc(R, RC)
    return _CACHE[key]


def run_cores(x2d, Wq, Wk, Wv, Wo, bo_v, R=None, RC=512, cores=None,
              **run_kwargs):
    """x2d: (ROWS, E) fp32.  Returns (ROWS, E) fp32."""
    ROWS = x2d.shape[0]
    if cores is None:
        cores = list(range(NCORE))
    n = len(cores)
    if R is None:
        R = ROWS // n
    assert R * n == ROWS
    nc = _get_nc(R, RC)

    bf = ml_dtypes.bfloat16
    scale = 1.0 / np.sqrt(np.sqrt(float(E)))  # fold E**-0.5 into both Wq, Wk
    wq_b = (Wq.astype(np.float64) * scale).astype(bf)
    wk_b = (Wk.astype(np.float64) * scale).astype(bf)
    wv_b = Wv.astype(bf)
    wo_b = Wo.astype(bf)
    bo_f = bo_v.reshape(1, E).astype(np.float32)

    in_maps = []
    for ci in range(n):
        xs = x2d[ci * R:(ci + 1) * R].T  # (E, R)
        in_maps.append({
            "xt": np.ascontiguousarray(xs).astype(bf),
            "wq": wq_b, "wk": wk_b, "wv": wv_b, "wo": wo_b,
        })
    res = run_bass_kernel_spmd(nc, in_maps, core_ids=cores, **run_kwargs)
    out = np.empty((ROWS, E), dtype=np.float32)
    for ci in range(n):
        ytd = res.results[ci]["yt"]  # (E, R), columns in natural row order
        out[ci * R:(ci + 1) * R] = ytd.T + bo_f  # bias added on host
    if run_kwargs.get("trace"):
        return out, res
    return out


def kernel(x, Wq, Wk, Wv, Wo, bo):
    x = np.asarray(x, dtype=np.float32)
    N, L, _ = x.shape
    y = run_cores(
        x.reshape(N * L, E),
        np.asarray(Wq, np.float32), np.asarray(Wk, np.float32),
        np.asarray(Wv, np.float32), np.asarray(Wo, np.float32),
        np.asarray(bo, np.float32),
    )
    return y.reshape(N, L, E)


# revision 25
# speedup vs baseline: 1.0477x; 1.0027x over previous
"""Trainium2 Bass kernel for the cross-head MultiHeadAttention module.

Reference computation (per row r of x flattened to (N*L, E)):
    q = x @ Wq; k = x @ Wk; v = x @ Wv           (E = 1024, H = 16, D = 64)
    energy[r, i, j] = sum_d q[r,i,d] * k[r,j,d]  (cross-head, per position)
    attn = softmax(energy / 32, axis=j)
    out[r, i, :] = sum_j attn[r,i,j] * v[r,j,:]
    y = out.reshape(R, E) @ Wo + bo

Distribution: data-parallel over rows (N*L = 16384 -> 2048 rows/core x 8).

Per-core design (all big matmuls in bf16 on the PE array), v5:
  *  Q/K projections run transposed (features on partitions, rows free).
     Their attention-layout rebuild (qd2b/kht2 block-diagonal slabs) is a
     pure partition-shift, so it runs as 32 direct SBUF->SBUF DMAs per
     tensor (512B runs both sides) with NO DRAM round trip; each DMA only
     depends on one projection chunk's psum->sbuf copy, so the rebuild
     streams behind the projection instead of after it.
  *  V runs natural (rows on partitions) and still round-trips through
     DRAM (its rebuild needs a partition<->free exchange which SBUF APs
     cannot express).  vd is double-buffered by pass parity so the
     readback issues immediately after the stage-out instead of waiting
     for the previous pass's AV matmuls (WAR removed).
  *  Energy: ONE matmul per row pair (pi, pi+RC/2): lhsT = qd2b[:, :, pi]
     ([128, 32] block-diagonal), rhs = kht2[:, :, pi] ([128, 16]); out is
     a dense [32, 16] block of a 256-row psum bank so softmax runs on
     dense [128, 512] tiles.
  *  softmax: exp (no max-subtraction: energies ~N(0, 1/16)), row-sum,
     reciprocal, scale+cast-to-bf16, 32x32-block vector transpose.
  *  A@V: ONE matmul per row pair; avp psum is 2 double-buffered 2-bank
     tiles (b-halves) so bank1's matmuls only wait on bank0's first-half
     extraction.
  *  y^T: full-width Wo matmuls accumulated in psum; + bo; DMA out.
  *  Schedule: the tensor queue interleaves pass p-1's attention with
     pass p's projections at chunk granularity:
       projq(p) | energy0(p-1) | projk(p) | energy1(p-1) | av0(p-1) |
       projv(p)+vd-readback(p) | av1(p-1) | Wo(p-1)
     so softmax / extraction / readback chains hide under projection
     matmuls and the PE p-state stays high.  Startup DMAs are split
     per-128-column chunk and ordered xtc -> Wq -> Wk -> Wv -> Wo.
"""

import numpy as np
import ml_dtypes

import concourse.bass as bass
from concourse import bacc
import concourse.tile as tile
from concourse import mybir
from concourse.bass_utils import run_bass_kernel_spmd

F32 = mybir.dt.float32
BF16 = mybir.dt.bfloat16
AF = mybir.ActivationFunctionType
ALU = mybir.AluOpType
AX = mybir.AxisListType

E = 1024
H = 16
D = 64
NCORE = 8


def build_nc(R, RC):
    """Per-core kernel program: R rows total, processed in passes of RC."""
    NP = R // RC          # passes
    NBK = RC // 256       # dense energy banks per pass (256 rows each)
    PH = RC // 2          # row pairs per pass

    nc = bacc.Bacc("TRN2", target_bir_lowering=False, debug=False)

    xt = nc.dram_tensor("xt", [E, R], BF16, kind="ExternalInput")
    wq = nc.dram_tensor("wq", [E, E], BF16, kind="ExternalInput")
    wk = nc.dram_tensor("wk", [E, E], BF16, kind="ExternalInput")
    wv = nc.dram_tensor("wv", [E, E], BF16, kind="ExternalInput")
    wo = nc.dram_tensor("wo", [E, E], BF16, kind="ExternalInput")
    yt = nc.dram_tensor("yt", [E, R], F32, kind="ExternalOutput")

    with tile.TileContext(nc) as tc:
        with (
            tc.tile_pool(name="wpool", bufs=1) as wpool,      # persistent
            tc.tile_pool(name="xpool", bufs=2) as xpool,      # xt chunks
            tc.tile_pool(name="spool", bufs=1) as spool,      # q/k/v staging
            tc.tile_pool(name="apool", bufs=2) as apool,      # softmax temps
            tc.tile_pool(name="opool", bufs=1) as opool,      # oft2
            tc.tile_pool(name="ypool", bufs=2) as ypool,      # y staging
            tc.tile_pool(name="dram", bufs=2, space="DRAM") as dpool,
            tc.tile_pool(name="pproj", bufs=2, space="PSUM") as pproj,
            tc.tile_pool(name="pe", bufs=2, space="PSUM") as pe_pool,
            tc.tile_pool(name="pav", bufs=2, space="PSUM") as pav,
        ):
            # ---- persistent loads (split per 128-col chunk; xtc(0) + wq
            # first so the first projection starts as early as possible).
            # DMA traffic classes: latency-critical relayouts (qk, vd) +
            # xtc go on the SP HWDGE (nc.sync); bulk streams (weights,
            # V stage-out, y out) go on the Activation HWDGE (nc.scalar)
            # so bulk descriptor storms never sit ahead of critical ones.
            wq_sb = wpool.tile([128, 8, E], BF16, tag="wq")
            wk_sb = wpool.tile([128, 8, E], BF16, tag="wk")
            wv_sb = wpool.tile([128, 8, E], BF16, tag="wv")
            wo_sb = wpool.tile([128, 8, E], BF16, tag="wo")

            xtr = xt.rearrange("(c p) r -> p c r", p=128)
            xtc0 = xpool.tile([128, 8, RC], BF16, tag="xtc")
            nc.sync.dma_start(xtc0[:], xtr[:, :, 0:RC])
            for w_sb, w_d in ((wq_sb, wq), (wk_sb, wk), (wv_sb, wv),
                              (wo_sb, wo)):
                nc.sync.dma_start(
                    w_sb[:], w_d.rearrange("(c p) e -> p c e", p=128))

            # block-diagonal operand tiles: zero blocks are memset once and
            # never rewritten (per-pass DMAs touch only the data blocks).
            # qd2b/kht2/vd ping-pong per pass parity so pass p+1's rebuild
            # DMAs never wait on pass p's attention matmuls.
            qd2bs, kht2s, vds = [], [], []
            for pp in range(2):
                qd2b_ = wpool.tile([128, 32, PH], BF16, tag=f"qd2b{pp}",
                                   name=f"qd2b{pp}")
                nc.vector.memset(qd2b_[0:64, 16:32, :], 0.0)
                nc.vector.memset(qd2b_[64:128, 0:16, :], 0.0)
                qd2bs.append(qd2b_)
                kht2_ = wpool.tile([128, 16, PH], BF16, tag=f"kht2{pp}",
                                   name=f"kht2{pp}")
                kht2s.append(kht2_)
                vd_ = wpool.tile([128, NBK * 32, 128], BF16, tag=f"vd{pp}",
                                 name=f"vd{pp}")
                nc.vector.memset(vd_[:], 0.0)
                vds.append(vd_)

            oft2 = opool.tile([128, 8, RC], BF16, tag="oft2")

            def proj_tr(name, w_sb, xtc):
                """Transposed projection (features on partitions)."""
                stg = spool.tile([128, 8, RC], BF16, tag=f"stg_{name}")
                for et in range(8):
                    ps = pproj.tile([128, RC], F32, tag="proj")
                    for c in range(8):
                        nc.tensor.matmul(
                            ps[:],
                            w_sb[:, c, et * 128:(et + 1) * 128],
                            xtc[:, c, :],
                            start=(c == 0),
                            stop=(c == 7),
                        )
                    if et % 2 == 0:
                        nc.vector.tensor_copy(stg[:, et, :], ps[:])
                    else:
                        nc.scalar.copy(stg[:, et, :], ps[:])
                return stg

            def relayout_q(it, stg):
                """stg_q -> qd2b[it%2] via 4 partition-shift SBUF DMAs.

                qd2b[64h'+d, 16h'+q, pi] = q^T[d, head q, row 256h'+pi]
                and stg[64(q%2)+d, q//2, r] = q^T[d, head q, row r], so
                each (h', q%2) group of 8 heads is one strided DMA.
                """
                qd2b_w = qd2bs[it % 2]
                for hp in range(2):
                    for hs in range(2):
                        nc.sync.dma_start(
                            qd2b_w[64 * hp:64 * hp + 64,
                                   16 * hp + hs:16 * hp + 16:2, :],
                            stg[64 * hs:64 * hs + 64, :,
                                256 * hp:256 * hp + 256],
                        )

            def relayout_k(it, stg):
                """stg_k -> kht2[it%2]: kht2[64h'+d, q, pi] = k^T[d, q,
                row 256h'+pi]."""
                kht2_w = kht2s[it % 2]
                for hp in range(2):
                    for hs in range(2):
                        nc.sync.dma_start(
                            kht2_w[64 * hp:64 * hp + 64, hs:16:2, :],
                            stg[64 * hs:64 * hs + 64, :,
                                256 * hp:256 * hp + 256],
                        )

            def proj_v(it, xtc):
                """Natural (row-major) V projection, staged to DRAM with
                rows permuted to (w, b, B, m, h) order so the vd readback
                needs only 8 DMA instructions.  Chunk rc_ holds the rows
                with (h, B) = divmod(rc_, NBK); stage DMAs use plain
                partition slices (16 calls, on the Activation HWDGE)."""
                vstg = spool.tile([128, RC // 128, E], BF16, tag="stg_v")
                v3d = dpool.tile([2, 4, NBK, 16, 2, E], BF16, tag="dram_v")
                for rc_ in range(RC // 128):
                    for h2 in range(2):
                        ps = pproj.tile([128, 512], F32, tag="proj")
                        for c in range(8):
                            nc.tensor.matmul(
                                ps[:],
                                xtc[:, c, rc_ * 128:(rc_ + 1) * 128],
                                wv_sb[:, c, h2 * 512:(h2 + 1) * 512],
                                start=(c == 0),
                                stop=(c == 7),
                            )
                        if h2 == 0:
                            nc.vector.tensor_copy(vstg[:, rc_, 0:512], ps[:])
                        else:
                            nc.scalar.copy(vstg[:, rc_, 512:1024], ps[:])
                    h, B = divmod(rc_, NBK)
                    for w in range(2):
                        for b in range(4):
                            nc.sync.dma_start(
                                v3d[w, b, B, :, h, :],
                                vstg[64 * w + 16 * b:64 * w + 16 * b + 16,
                                     rc_, :],
                            )
                return v3d

            def vd_readback(it, v3d):
                """DRAM -> vd[it%2] block-diagonal V slabs: one DMA per
                (row-half w2, quarter b), 3-dim APs on both sides."""
                vd_w = vds[it % 2]
                for w2 in range(2):
                    for b in range(4):
                        nc.sync.dma_start(
                            vd_w[32 * b + 16 * w2:32 * b + 16 * w2 + 16,
                                 :, 64 * w2:64 * w2 + 64],
                            v3d[w2, b].rearrange(
                                "B m h (j d) -> j (B m h) d", j=16, d=64),
                        )

            def energy_softmax(B, par):
                """One dense 256-row energy bank + its softmax; returns att."""
                qd2b, kht2 = qd2bs[par], kht2s[par]
                ep = pe_pool.tile([128, 32, 16], F32, tag="ep")
                for lam in range(128):
                    pi = 128 * B + lam
                    b, s = (lam // 16) % 4, 2 * (lam % 16) + lam // 64
                    nc.tensor.matmul(
                        ep[32 * b:32 * b + 32, s, :],
                        qd2b[:, :, pi],
                        kht2[:, :, pi],
                        start=True,
                        stop=True,
                        tile_position=(0, 32 * b),
                    )
                ex = apool.tile([128, 32, 16], F32, tag="ex")
                nc.scalar.activation(ex[:], ep[:], AF.Exp)
                sm = apool.tile([128, 32], F32, tag="sm")
                nc.vector.reduce_sum(sm[:], ex[:], axis=AX.X)
                rcp = apool.tile([128, 32], F32, tag="rcp")
                nc.vector.reciprocal(rcp[:], sm[:])
                at = apool.tile([128, 32, 16], BF16, tag="at")
                nc.vector.tensor_tensor(
                    at[:], ex[:],
                    rcp[:, :, None].to_broadcast([128, 32, 16]),
                    ALU.mult,
                )
                att = apool.tile([128, 512], BF16, tag="att")
                nc.vector.transpose(att[:], at[:].rearrange("p a b -> p (a b)"))
                return att

            def av_extract(B, att, par):
                """A@V for bank B: two 2-bank psum tiles (b-halves) + 8
                merged extraction copies."""
                vd = vds[par]
                dstx = oft2[:].rearrange(
                    "p g (h Bk wc) -> p g h Bk wc", h=2, Bk=NBK)
                for bh in range(2):
                    avp = pav.tile([128, 2, 32, 16], F32, tag="avp")
                    for b2 in range(2):
                        b = 2 * bh + b2
                        for t in range(32):
                            nc.tensor.matmul(
                                avp[:, b2, t, :],
                                vd[32 * b:32 * b + 32, 32 * B + t, :],
                                att[32 * b:32 * b + 32, 16 * t:16 * t + 16],
                                start=True,
                                stop=True,
                                tile_position=(32 * b, 0),
                            )
                    # avp[64w+d, b2, 2m+rho, q] -> oft2[64(q%2)+d, q//2,
                    #   256B + 128rho + 64w + 32bh + 16b2 + m]
                    srcx = avp[:].rearrange(
                        "p b (m r) (g s) -> p g r (b m) s", r=2, s=2)
                    for w in range(2):
                        for sg in range(2):
                            srcc = srcx[64 * w:64 * w + 64, :, :, :, sg]
                            dst = dstx[64 * sg:64 * sg + 64, :, :, B,
                                       64 * w + 32 * bh:64 * w + 32 * bh + 32]
                            if (w + sg + B + bh) % 2 == 0:
                                nc.vector.tensor_copy(dst, srcc)
                            else:
                                nc.scalar.copy(dst, srcc)

            def wo_out(p):
                """y^T = Wo^T-chunks @ oft2, DMA out (bias added on host)."""
                for c in range(8):
                    # rotates through the ep buffers (attention is done
                    # with them by now) -> double-buffered Wo psum at no
                    # extra bank cost
                    ytp = pe_pool.tile([128, RC], F32, tag="ep")
                    for g in range(8):
                        nc.tensor.matmul(
                            ytp[:],
                            wo_sb[:, g, 128 * c:128 * c + 128],
                            oft2[:, g, :],
                            start=(g == 0),
                            stop=(g == 7),
                        )
                    ys = ypool.tile([128, RC], F32, tag="ys")
                    if c % 2 == 0:
                        nc.vector.tensor_copy(ys[:], ytp[:])
                    else:
                        nc.scalar.copy(ys[:], ytp[:])
                    nc.scalar.dma_start(
                        yt.rearrange("(t q) r -> q t r", q=128)[
                            :, c, p * RC:(p + 1) * RC
                        ],
                        ys[:],
                    )

            # Depth-2 software pipeline: attention for pass p runs during
            # iteration p+1, interleaved with pass p+1's projections at
            # chunk granularity on the tensor queue.  All rebuild targets
            # (qd2b/kht2/vd) ping-pong on pass parity, so iteration it's
            # rebuilds (parity it%2) never touch the buffers pass it-1's
            # attention is reading (parity (it-1)%2).
            xtcs = {0: xtc0}
            for it in range(NP + 1):
                do_proj = it < NP
                do_att = it >= 1
                p = it - 1
                if do_proj:
                    xtc = xtcs.pop(it)
                    # prefetch next pass's x chunk a full iteration ahead
                    if it + 1 < NP:
                        r0 = (it + 1) * RC
                        xn = xpool.tile([128, 8, RC], BF16, tag="xtc")
                        nc.sync.dma_start(xn[:], xtr[:, :, r0:r0 + RC])
                        xtcs[it + 1] = xn

                # --- interleaved tensor-queue schedule ---
                if do_proj:
                    q_stg = proj_tr("q", wq_sb, xtc)
                    relayout_q(it, q_stg)
                if do_att:
                    att0 = energy_softmax(0, p % 2)
                if do_proj:
                    k_stg = proj_tr("k", wk_sb, xtc)
                    relayout_k(it, k_stg)
                if do_att:
                    att1 = energy_softmax(1, p % 2)
                    av_extract(0, att0, p % 2)
                if do_proj:
                    v3d = proj_v(it, xtc)
                    vd_readback(it, v3d)
                if do_att:
                    av_extract(1, att1, p % 2)
                    wo_out(p)

    nc.finalize()
    return nc


_CACHE = {}


def _get_nc(R, RC):
    key = (R, RC)
    if key not in _CACHE:
        _CACHE[key] = build_nc(R, RC)
    return _CACHE[key]


def run_cores(x2d, Wq, Wk, Wv, Wo, bo_v, R=None, RC=512, cores=None,
              **run_kwargs):
    """x2d: (ROWS, E) fp32.  Returns (ROWS, E) fp32."""
    ROWS = x2d.shape[0]
    if cores is None:
        cores = list(range(NCORE))
    n = len(cores)
    if R is None:
        R = ROWS // n
    assert R * n == ROWS
    nc = _get_nc(R, RC)

    bf = ml_dtypes.bfloat16
    scale = 1.0 / np.sqrt(np.sqrt(float(E)))  # fold E**-0.5 into both Wq, Wk
    wq_b = (Wq.astype(np.float64) * scale).astype(bf)
    wk_b = (Wk.astype(np.float64) * scale).astype(bf)
    wv_b = Wv.astype(bf)
    wo_b = Wo.astype(bf)
    bo_f = bo_v.reshape(1, E).astype(np.float32)

    in_maps = []
    for ci in range(n):
        xs = x2d[ci * R:(ci + 1) * R].T  # (E, R)
        in_maps.append({
            "xt": np.ascontiguousarray(xs).astype(bf),
            "wq": wq_b, "wk": wk_b, "wv": wv_b, "wo": wo_b,
        })
    res = run_bass_kernel_spmd(nc, in_maps, core_ids=cores, **run_kwargs)
    out = np.empty((ROWS, E), dtype=np.float32)
    for ci in range(n):
        ytd = res.results[ci]["yt"]  # (E, R), columns in natural row order
        out[ci * R:(ci + 1) * R] = ytd.T + bo_f  # bias added on host
    if run_kwargs.get("trace"):
        return out, res
    return out


def kernel(x, Wq, Wk, Wv, Wo, bo):
    x = np.asarray(x, dtype=np.float32)
    N, L, _ = x.shape
    y = run_cores(
        x.reshape(N * L, E),
        np.asarray(Wq, np.float32), np.asarray(Wk, np.float32),
        np.asarray(Wv, np.float32), np.asarray(Wo, np.float32),
        np.asarray(bo, np.float32),
    )
    return y.reshape(N, L, E)


# revision 26
# speedup vs baseline: 1.1571x; 1.1043x over previous
"""Trainium2 Bass kernel for the cross-head MultiHeadAttention module.

Reference computation (per row r of x flattened to (N*L, E)):
    q = x @ Wq; k = x @ Wk; v = x @ Wv           (E = 1024, H = 16, D = 64)
    energy[r, i, j] = sum_d q[r,i,d] * k[r,j,d]  (cross-head, per position)
    attn = softmax(energy / 32, axis=j)
    out[r, i, :] = sum_j attn[r,i,j] * v[r,j,:]
    y = out.reshape(R, E) @ Wo + bo

Distribution: data-parallel over rows (N*L = 16384 -> 2048 rows/core x 8).

Per-core design (all big matmuls in bf16 on the PE array), v5:
  *  Q/K projections run transposed (features on partitions, rows free).
     Their attention-layout rebuild (qd2b/kht2 block-diagonal slabs) is a
     pure partition-shift, so it runs as 32 direct SBUF->SBUF DMAs per
     tensor (512B runs both sides) with NO DRAM round trip; each DMA only
     depends on one projection chunk's psum->sbuf copy, so the rebuild
     streams behind the projection instead of after it.
  *  V runs natural (rows on partitions) and still round-trips through
     DRAM (its rebuild needs a partition<->free exchange which SBUF APs
     cannot express).  vd is double-buffered by pass parity so the
     readback issues immediately after the stage-out instead of waiting
     for the previous pass's AV matmuls (WAR removed).
  *  Energy: ONE matmul per row pair (pi, pi+RC/2): lhsT = qd2b[:, :, pi]
     ([128, 32] block-diagonal), rhs = kht2[:, :, pi] ([128, 16]); out is
     a dense [32, 16] block of a 256-row psum bank so softmax runs on
     dense [128, 512] tiles.
  *  softmax: exp (no max-subtraction: energies ~N(0, 1/16)), row-sum,
     reciprocal, scale+cast-to-bf16, 32x32-block vector transpose.
  *  A@V: ONE matmul per row pair; avp psum is 2 double-buffered 2-bank
     tiles (b-halves) so bank1's matmuls only wait on bank0's first-half
     extraction.
  *  y^T: full-width Wo matmuls accumulated in psum; + bo; DMA out.
  *  Schedule: the tensor queue interleaves pass p-1's attention with
     pass p's projections at chunk granularity:
       projq(p) | energy0(p-1) | projk(p) | energy1(p-1) | av0(p-1) |
       projv(p)+vd-readback(p) | av1(p-1) | Wo(p-1)
     so softmax / extraction / readback chains hide under projection
     matmuls and the PE p-state stays high.  Startup DMAs are split
     per-128-column chunk and ordered xtc -> Wq -> Wk -> Wv -> Wo.
"""

import numpy as np
import ml_dtypes

import concourse.bass as bass
from concourse import bacc
import concourse.tile as tile
from concourse import mybir
from concourse.bass_utils import run_bass_kernel_spmd

F32 = mybir.dt.float32
BF16 = mybir.dt.bfloat16
AF = mybir.ActivationFunctionType
ALU = mybir.AluOpType
AX = mybir.AxisListType

E = 1024
H = 16
D = 64
NCORE = 8


def build_nc(R, RC):
    """Per-core kernel program: R rows total, processed in passes of RC."""
    NP = R // RC          # passes
    NBK = RC // 256       # dense energy banks per pass (256 rows each)
    PH = RC // 2          # row pairs per pass

    nc = bacc.Bacc("TRN2", target_bir_lowering=False, debug=False)

    xt = nc.dram_tensor("xt", [E, R], BF16, kind="ExternalInput")
    wq = nc.dram_tensor("wq", [E, E], BF16, kind="ExternalInput")
    wk = nc.dram_tensor("wk", [E, E], BF16, kind="ExternalInput")
    wv = nc.dram_tensor("wv", [E, E], BF16, kind="ExternalInput")
    wo = nc.dram_tensor("wo", [E, E], BF16, kind="ExternalInput")
    yt = nc.dram_tensor("yt", [E, R], F32, kind="ExternalOutput")

    with tile.TileContext(nc) as tc:
        with (
            tc.tile_pool(name="wpool", bufs=1) as wpool,      # persistent
            tc.tile_pool(name="xpool", bufs=2) as xpool,      # xt chunks
            tc.tile_pool(name="spool", bufs=1) as spool,      # q/k/v staging
            tc.tile_pool(name="apool", bufs=2) as apool,      # softmax temps
            tc.tile_pool(name="opool", bufs=1) as opool,      # oft2
            tc.tile_pool(name="ypool", bufs=2) as ypool,      # y staging
            tc.tile_pool(name="dram", bufs=2, space="DRAM") as dpool,
            tc.tile_pool(name="pproj", bufs=2, space="PSUM") as pproj,
            tc.tile_pool(name="pe", bufs=2, space="PSUM") as pe_pool,
            tc.tile_pool(name="pav", bufs=2, space="PSUM") as pav,
        ):
            # ---- persistent loads (split per 128-col chunk; xtc(0) + wq
            # first so the first projection starts as early as possible).
            # DMA traffic classes: latency-critical relayouts (qk, vd) +
            # xtc go on the SP HWDGE (nc.sync); bulk streams (weights,
            # V stage-out, y out) go on the Activation HWDGE (nc.scalar)
            # so bulk descriptor storms never sit ahead of critical ones.
            wq_sb = wpool.tile([128, 8, E], BF16, tag="wq")
            wk_sb = wpool.tile([128, 8, E], BF16, tag="wk")
            wv_sb = wpool.tile([128, 8, E], BF16, tag="wv")
            wo_sb = wpool.tile([128, 8, E], BF16, tag="wo")

            xtr = xt.rearrange("(c p) r -> p c r", p=128)
            xtc0 = xpool.tile([128, 8, RC], BF16, tag="xtc")
            for c in range(8):
                nc.sync.dma_start(xtc0[:, c, :], xtr[:, c, 0:RC])
            wvr = wv.rearrange("(c p) e -> p c e", p=128)
            for c in range(8):
                nc.sync.dma_start(wv_sb[:, c, :], wvr[:, c, :])
            wqr = wq.rearrange("(c p) e -> p c e", p=128)
            for c in range(8):
                nc.sync.dma_start(wq_sb[:, c, :], wqr[:, c, :])
            nc.sync.dma_start(
                wk_sb[:], wk.rearrange("(c p) e -> p c e", p=128))
            nc.sync.dma_start(
                wo_sb[:], wo.rearrange("(c p) e -> p c e", p=128))

            # block-diagonal operand tiles: zero blocks are memset once and
            # never rewritten (per-pass DMAs touch only the data blocks).
            # qd2b/kht2/vd ping-pong per pass parity so pass p+1's rebuild
            # DMAs never wait on pass p's attention matmuls.
            qd2bs, kht2s, vds = [], [], []
            for pp in range(2):
                qd2b_ = wpool.tile([128, 32, PH], BF16, tag=f"qd2b{pp}",
                                   name=f"qd2b{pp}")
                nc.vector.memset(qd2b_[0:64, 16:32, :], 0.0)
                nc.vector.memset(qd2b_[64:128, 0:16, :], 0.0)
                qd2bs.append(qd2b_)
                kht2_ = wpool.tile([128, 16, PH], BF16, tag=f"kht2{pp}",
                                   name=f"kht2{pp}")
                kht2s.append(kht2_)
                vd_ = wpool.tile([128, NBK * 32, 128], BF16, tag=f"vd{pp}",
                                 name=f"vd{pp}")
                nc.vector.memset(vd_[:], 0.0)
                vds.append(vd_)

            oft2 = opool.tile([128, 8, RC], BF16, tag="oft2")

            def proj_tr(name, w_sb, xtc):
                """Transposed projection (features on partitions)."""
                stg = spool.tile([128, 8, RC], BF16, tag=f"stg_{name}")
                for et in range(8):
                    ps = pproj.tile([128, RC], F32, tag="proj")
                    for c in range(8):
                        nc.tensor.matmul(
                            ps[:],
                            w_sb[:, c, et * 128:(et + 1) * 128],
                            xtc[:, c, :],
                            start=(c == 0),
                            stop=(c == 7),
                        )
                    if et % 2 == 0:
                        nc.vector.tensor_copy(stg[:, et, :], ps[:])
                    else:
                        nc.scalar.copy(stg[:, et, :], ps[:])
                return stg

            def relayout_q(it, stg):
                """stg_q -> qd2b[it%2] via 4 partition-shift SBUF DMAs.

                qd2b[64h'+d, 16h'+q, pi] = q^T[d, head q, row 256h'+pi]
                and stg[64(q%2)+d, q//2, r] = q^T[d, head q, row r], so
                each (h', q%2) group of 8 heads is one strided DMA.
                """
                qd2b_w = qd2bs[it % 2]
                for hp in range(2):
                    for hs in range(2):
                        nc.sync.dma_start(
                            qd2b_w[64 * hp:64 * hp + 64,
                                   16 * hp + hs:16 * hp + 16:2, :],
                            stg[64 * hs:64 * hs + 64, :,
                                256 * hp:256 * hp + 256],
                        )

            def relayout_k(it, stg):
                """stg_k -> kht2[it%2]: kht2[64h'+d, q, pi] = k^T[d, q,
                row 256h'+pi]."""
                kht2_w = kht2s[it % 2]
                for hp in range(2):
                    for hs in range(2):
                        nc.sync.dma_start(
                            kht2_w[64 * hp:64 * hp + 64, hs:16:2, :],
                            stg[64 * hs:64 * hs + 64, :,
                                256 * hp:256 * hp + 256],
                        )

            def proj_v(it, xtc):
                """Natural (row-major) V projection, staged to DRAM with
                rows permuted to (w, b, B, m, h) order so the vd readback
                needs only 8 DMA instructions.  Chunk rc_ holds the rows
                with (h, B) = divmod(rc_, NBK); stage DMAs use plain
                partition slices (16 calls, on the Activation HWDGE)."""
                vstg = spool.tile([128, RC // 128, E], BF16, tag="stg_v")
                v3d = dpool.tile([2, 4, NBK, 16, 2, E], BF16, tag="dram_v")
                for rc_ in range(RC // 128):
                    for h2 in range(2):
                        ps = pproj.tile([128, 512], F32, tag="proj")
                        for c in range(8):
                            nc.tensor.matmul(
                                ps[:],
                                xtc[:, c, rc_ * 128:(rc_ + 1) * 128],
                                wv_sb[:, c, h2 * 512:(h2 + 1) * 512],
                                start=(c == 0),
                                stop=(c == 7),
                            )
                        if h2 == 0:
                            nc.vector.tensor_copy(vstg[:, rc_, 0:512], ps[:])
                        else:
                            nc.scalar.copy(vstg[:, rc_, 512:1024], ps[:])
                    h, B = divmod(rc_, NBK)
                    for w in range(2):
                        for b in range(4):
                            nc.sync.dma_start(
                                v3d[w, b, B, :, h, :],
                                vstg[64 * w + 16 * b:64 * w + 16 * b + 16,
                                     rc_, :],
                            )
                return v3d

            def vd_readback(it, v3d):
                """DRAM -> vd[it%2] block-diagonal V slabs: one DMA per
                (row-half w2, quarter b), 3-dim APs on both sides."""
                vd_w = vds[it % 2]
                for w2 in range(2):
                    for b in range(4):
                        nc.sync.dma_start(
                            vd_w[32 * b + 16 * w2:32 * b + 16 * w2 + 16,
                                 :, 64 * w2:64 * w2 + 64],
                            v3d[w2, b].rearrange(
                                "B m h (j d) -> j (B m h) d", j=16, d=64),
                        )

            def energy_softmax(B, par):
                """One dense 256-row energy bank + its softmax; returns att."""
                qd2b, kht2 = qd2bs[par], kht2s[par]
                ep = pe_pool.tile([128, 32, 16], F32, tag="ep")
                for lam in range(128):
                    pi = 128 * B + lam
                    b, s = (lam // 16) % 4, 2 * (lam % 16) + lam // 64
                    nc.tensor.matmul(
                        ep[32 * b:32 * b + 32, s, :],
                        qd2b[:, :, pi],
                        kht2[:, :, pi],
                        start=True,
                        stop=True,
                        tile_position=(0, 32 * b),
                    )
                ex = apool.tile([128, 32, 16], F32, tag="ex")
                nc.scalar.activation(ex[:], ep[:], AF.Exp)
                sm = apool.tile([128, 32], F32, tag="sm")
                nc.vector.reduce_sum(sm[:], ex[:], axis=AX.X)
                rcp = apool.tile([128, 32], F32, tag="rcp")
                nc.vector.reciprocal(rcp[:], sm[:])
                at = apool.tile([128, 32, 16], BF16, tag="at")
                nc.vector.tensor_tensor(
                    at[:], ex[:],
                    rcp[:, :, None].to_broadcast([128, 32, 16]),
                    ALU.mult,
                )
                att = apool.tile([128, 512], BF16, tag="att")
                nc.vector.transpose(att[:], at[:].rearrange("p a b -> p (a b)"))
                return att

            def av_extract(B, att, par):
                """A@V for bank B: two 2-bank psum tiles (b-halves) + 8
                merged extraction copies."""
                vd = vds[par]
                dstx = oft2[:].rearrange(
                    "p g (h Bk wc) -> p g h Bk wc", h=2, Bk=NBK)
                for bh in range(2):
                    avp = pav.tile([128, 2, 32, 16], F32, tag="avp")
                    for b2 in range(2):
                        b = 2 * bh + b2
                        for t in range(32):
                            nc.tensor.matmul(
                                avp[:, b2, t, :],
                                vd[32 * b:32 * b + 32, 32 * B + t, :],
                                att[32 * b:32 * b + 32, 16 * t:16 * t + 16],
                                start=True,
                                stop=True,
                                tile_position=(32 * b, 0),
                            )
                    # avp[64w+d, b2, 2m+rho, q] -> oft2[64(q%2)+d, q//2,
                    #   256B + 128rho + 64w + 32bh + 16b2 + m]
                    srcx = avp[:].rearrange(
                        "p b (m r) (g s) -> p g r (b m) s", r=2, s=2)
                    for w in range(2):
                        for sg in range(2):
                            srcc = srcx[64 * w:64 * w + 64, :, :, :, sg]
                            dst = dstx[64 * sg:64 * sg + 64, :, :, B,
                                       64 * w + 32 * bh:64 * w + 32 * bh + 32]
                            if (w + sg + B + bh) % 2 == 0:
                                nc.vector.tensor_copy(dst, srcc)
                            else:
                                nc.scalar.copy(dst, srcc)

            def wo_out(p):
                """y^T = Wo^T-chunks @ oft2, DMA out (bias added on host)."""
                for c in range(8):
                    # rotates through the ep buffers (attention is done
                    # with them by now) -> double-buffered Wo psum at no
                    # extra bank cost
                    ytp = pe_pool.tile([128, RC], F32, tag="ep")
                    for g in range(8):
                        nc.tensor.matmul(
                            ytp[:],
                            wo_sb[:, g, 128 * c:128 * c + 128],
                            oft2[:, g, :],
                            start=(g == 0),
                            stop=(g == 7),
                        )
                    ys = ypool.tile([128, RC], F32, tag="ys")
                    if c % 2 == 0:
                        nc.vector.tensor_copy(ys[:], ytp[:])
                    else:
                        nc.scalar.copy(ys[:], ytp[:])
                    nc.sync.dma_start(
                        yt.rearrange("(t q) r -> q t r", q=128)[
                            :, c, p * RC:(p + 1) * RC
                        ],
                        ys[:],
                    )

            # Depth-2 software pipeline: attention for pass p runs during
            # iteration p+1, interleaved with pass p+1's projections at
            # chunk granularity on the tensor queue.  All rebuild targets
            # (qd2b/kht2/vd) ping-pong on pass parity, so iteration it's
            # rebuilds (parity it%2) never touch the buffers pass it-1's
            # attention is reading (parity (it-1)%2).
            xtcs = {0: xtc0}
            for it in range(NP + 1):
                do_proj = it < NP
                do_att = it >= 1
                p = it - 1
                if do_proj:
                    xtc = xtcs.pop(it)
                    # prefetch next pass's x chunk a full iteration ahead
                    if it + 1 < NP:
                        r0 = (it + 1) * RC
                        xn = xpool.tile([128, 8, RC], BF16, tag="xtc")
                        nc.sync.dma_start(xn[:], xtr[:, :, r0:r0 + RC])
                        xtcs[it + 1] = xn

                # --- interleaved tensor-queue schedule (V first so its
                # DRAM round trip completes mid-iteration) ---
                if do_proj:
                    v3d = proj_v(it, xtc)
                    vd_readback(it, v3d)
                if do_att:
                    att0 = energy_softmax(0, p % 2)
                if do_proj:
                    q_stg = proj_tr("q", wq_sb, xtc)
                    relayout_q(it, q_stg)
                if do_att:
                    att1 = energy_softmax(1, p % 2)
                    av_extract(0, att0, p % 2)
                if do_proj:
                    k_stg = proj_tr("k", wk_sb, xtc)
                    relayout_k(it, k_stg)
                if do_att:
                    av_extract(1, att1, p % 2)
                    wo_out(p)

    nc.finalize()
    return nc


_CACHE = {}


def _get_nc(R, RC):
    key = (R, RC)
    if key not in _CACHE:
        _CACHE[key] = build_nc(R, RC)
    return _CACHE[key]


def run_cores(x2d, Wq, Wk, Wv, Wo, bo_v, R=None, RC=512, cores=None,
              **run_kwargs):
    """x2d: (ROWS, E) fp32.  Returns (ROWS, E) fp32."""
    ROWS = x2d.shape[0]
    if cores is None:
        cores = list(range(NCORE))
    n = len(cores)
    if R is None:
        R = ROWS // n
    assert R * n == ROWS
    nc = _get_nc(R, RC)

    bf = ml_dtypes.bfloat16
    scale = 1.0 / np.sqrt(np.sqrt(float(E)))  # fold E**-0.5 into both Wq, Wk
    wq_b = (Wq.astype(np.float64) * scale).astype(bf)
    wk_b = (Wk.astype(np.float64) * scale).astype(bf)
    wv_b = Wv.astype(bf)
    wo_b = Wo.astype(bf)
    bo_f = bo_v.reshape(1, E).astype(np.float32)

    in_maps = []
    for ci in range(n):
        xs = x2d[ci * R:(ci + 1) * R].T  # (E, R)
        in_maps.append({
            "xt": np.ascontiguousarray(xs).astype(bf),
            "wq": wq_b, "wk": wk_b, "wv": wv_b, "wo": wo_b,
        })
    res = run_bass_kernel_spmd(nc, in_maps, core_ids=cores, **run_kwargs)
    out = np.empty((ROWS, E), dtype=np.float32)
    for ci in range(n):
        ytd = res.results[ci]["yt"]  # (E, R), columns in natural row order
        out[ci * R:(ci + 1) * R] = ytd.T + bo_f  # bias added on host
    if run_kwargs.get("trace"):
        return out, res
    return out


def kernel(x, Wq, Wk, Wv, Wo, bo):
    x = np.asarray(x, dtype=np.float32)
    N, L, _ = x.shape
    y = run_cores(
        x.reshape(N * L, E),
        np.asarray(Wq, np.float32), np.asarray(Wk, np.float32),
        np.asarray(Wv, np.float32), np.asarray(Wo, np.float32),
        np.asarray(bo, np.float32),
    )
    return y.reshape(N, L, E)


# revision 27
# speedup vs baseline: 1.1788x; 1.0188x over previous
"""Trainium2 Bass kernel for the cross-head MultiHeadAttention module.

Reference computation (per row r of x flattened to (N*L, E)):
    q = x @ Wq; k = x @ Wk; v = x @ Wv           (E = 1024, H = 16, D = 64)
    energy[r, i, j] = sum_d q[r,i,d] * k[r,j,d]  (cross-head, per position)
    attn = softmax(energy / 32, axis=j)
    out[r, i, :] = sum_j attn[r,i,j] * v[r,j,:]
    y = out.reshape(R, E) @ Wo + bo

Distribution: data-parallel over rows (N*L = 16384 -> 2048 rows/core x 8).

Per-core design (all big matmuls in bf16 on the PE array), v5:
  *  Q/K projections run transposed (features on partitions, rows free).
     Their attention-layout rebuild (qd2b/kht2 block-diagonal slabs) is a
     pure partition-shift, so it runs as 32 direct SBUF->SBUF DMAs per
     tensor (512B runs both sides) with NO DRAM round trip; each DMA only
     depends on one projection chunk's psum->sbuf copy, so the rebuild
     streams behind the projection instead of after it.
  *  V runs natural (rows on partitions) and still round-trips through
     DRAM (its rebuild needs a partition<->free exchange which SBUF APs
     cannot express).  vd is double-buffered by pass parity so the
     readback issues immediately after the stage-out instead of waiting
     for the previous pass's AV matmuls (WAR removed).
  *  Energy: ONE matmul per row pair (pi, pi+RC/2): lhsT = qd2b[:, :, pi]
     ([128, 32] block-diagonal), rhs = kht2[:, :, pi] ([128, 16]); out is
     a dense [32, 16] block of a 256-row psum bank so softmax runs on
     dense [128, 512] tiles.
  *  softmax: exp (no max-subtraction: energies ~N(0, 1/16)), row-sum,
     reciprocal, scale+cast-to-bf16, 32x32-block vector transpose.
  *  A@V: ONE matmul per row pair; avp psum is 2 double-buffered 2-bank
     tiles (b-halves) so bank1's matmuls only wait on bank0's first-half
     extraction.
  *  y^T: full-width Wo matmuls accumulated in psum; + bo; DMA out.
  *  Schedule: the tensor queue interleaves pass p-1's attention with
     pass p's projections at chunk granularity:
       projq(p) | energy0(p-1) | projk(p) | energy1(p-1) | av0(p-1) |
       projv(p)+vd-readback(p) | av1(p-1) | Wo(p-1)
     so softmax / extraction / readback chains hide under projection
     matmuls and the PE p-state stays high.  Startup DMAs are split
     per-128-column chunk and ordered xtc -> Wq -> Wk -> Wv -> Wo.
"""

import numpy as np
import ml_dtypes

import concourse.bass as bass
from concourse import bacc
import concourse.tile as tile
from concourse import mybir
from concourse.bass_utils import run_bass_kernel_spmd

F32 = mybir.dt.float32
BF16 = mybir.dt.bfloat16
AF = mybir.ActivationFunctionType
ALU = mybir.AluOpType
AX = mybir.AxisListType

E = 1024
H = 16
D = 64
NCORE = 8


def build_nc(R, RC):
    """Per-core kernel program: R rows total, processed in passes of RC."""
    NP = R // RC          # passes
    NBK = RC // 256       # dense energy banks per pass (256 rows each)
    PH = RC // 2          # row pairs per pass

    nc = bacc.Bacc("TRN2", target_bir_lowering=False, debug=False)

    xt = nc.dram_tensor("xt", [E, R], BF16, kind="ExternalInput")
    wq = nc.dram_tensor("wq", [E, E], BF16, kind="ExternalInput")
    wk = nc.dram_tensor("wk", [E, E], BF16, kind="ExternalInput")
    wv = nc.dram_tensor("wv", [E, E], BF16, kind="ExternalInput")
    wo = nc.dram_tensor("wo", [E, E], BF16, kind="ExternalInput")
    yt = nc.dram_tensor("yt", [E, R], F32, kind="ExternalOutput")

    with tile.TileContext(nc) as tc:
        with (
            tc.tile_pool(name="wpool", bufs=1) as wpool,      # persistent
            tc.tile_pool(name="xpool", bufs=2) as xpool,      # xt chunks
            tc.tile_pool(name="spool", bufs=1) as spool,      # q/k/v staging
            tc.tile_pool(name="apool", bufs=2) as apool,      # softmax temps
            tc.tile_pool(name="opool", bufs=1) as opool,      # oft2
            tc.tile_pool(name="ypool", bufs=2) as ypool,      # y staging
            tc.tile_pool(name="dram", bufs=2, space="DRAM") as dpool,
            tc.tile_pool(name="pproj", bufs=2, space="PSUM") as pproj,
            tc.tile_pool(name="pe", bufs=2, space="PSUM") as pe_pool,
            tc.tile_pool(name="pav", bufs=2, space="PSUM") as pav,
        ):
            # ---- persistent loads (split per 128-col chunk; xtc(0) + wq
            # first so the first projection starts as early as possible).
            # DMA traffic classes: latency-critical relayouts (qk, vd) +
            # xtc go on the SP HWDGE (nc.sync); bulk streams (weights,
            # V stage-out, y out) go on the Activation HWDGE (nc.scalar)
            # so bulk descriptor storms never sit ahead of critical ones.
            wq_sb = wpool.tile([128, 8, E], BF16, tag="wq")
            wk_sb = wpool.tile([128, 8, E], BF16, tag="wk")
            wv_sb = wpool.tile([128, 8, E], BF16, tag="wv")
            wo_sb = wpool.tile([128, 8, E], BF16, tag="wo")

            xtr = xt.rearrange("(c p) r -> p c r", p=128)
            xtc0 = xpool.tile([128, 8, RC], BF16, tag="xtc")
            for c in range(8):
                nc.sync.dma_start(xtc0[:, c, :], xtr[:, c, 0:RC])
            wvr = wv.rearrange("(c p) e -> p c e", p=128)
            for c in range(8):
                nc.sync.dma_start(wv_sb[:, c, :], wvr[:, c, :])
            wqr = wq.rearrange("(c p) e -> p c e", p=128)
            for c in range(8):
                nc.sync.dma_start(wq_sb[:, c, :], wqr[:, c, :])
            nc.sync.dma_start(
                wk_sb[:], wk.rearrange("(c p) e -> p c e", p=128))
            nc.sync.dma_start(
                wo_sb[:], wo.rearrange("(c p) e -> p c e", p=128))

            # block-diagonal operand tiles: zero blocks are memset once and
            # never rewritten (per-pass DMAs touch only the data blocks).
            # qd2b/kht2/vd ping-pong per pass parity so pass p+1's rebuild
            # DMAs never wait on pass p's attention matmuls.
            qd2bs, kht2s, vds = [], [], []
            for pp in range(2):
                qd2b_ = wpool.tile([128, 32, PH], BF16, tag=f"qd2b{pp}",
                                   name=f"qd2b{pp}")
                nc.gpsimd.memset(qd2b_[0:64, 16:32, :], 0.0)
                nc.gpsimd.memset(qd2b_[64:128, 0:16, :], 0.0)
                qd2bs.append(qd2b_)
                kht2_ = wpool.tile([128, 16, PH], BF16, tag=f"kht2{pp}",
                                   name=f"kht2{pp}")
                kht2s.append(kht2_)
                vd_ = wpool.tile([128, NBK * 32, 128], BF16, tag=f"vd{pp}",
                                 name=f"vd{pp}")
                nc.gpsimd.memset(vd_[:], 0.0)
                vds.append(vd_)

            oft2 = opool.tile([128, 8, RC], BF16, tag="oft2")

            def proj_tr(name, w_sb, xtc):
                """Transposed projection (features on partitions)."""
                stg = spool.tile([128, 8, RC], BF16, tag=f"stg_{name}")
                for et in range(8):
                    ps = pproj.tile([128, RC], F32, tag="proj")
                    for c in range(8):
                        nc.tensor.matmul(
                            ps[:],
                            w_sb[:, c, et * 128:(et + 1) * 128],
                            xtc[:, c, :],
                            start=(c == 0),
                            stop=(c == 7),
                        )
                    if et % 2 == 0:
                        nc.vector.tensor_copy(stg[:, et, :], ps[:])
                    else:
                        nc.scalar.copy(stg[:, et, :], ps[:])
                return stg

            def relayout_q(it, stg):
                """stg_q -> qd2b[it%2] via 4 partition-shift SBUF DMAs.

                qd2b[64h'+d, 16h'+q, pi] = q^T[d, head q, row 256h'+pi]
                and stg[64(q%2)+d, q//2, r] = q^T[d, head q, row r], so
                each (h', q%2) group of 8 heads is one strided DMA.
                """
                qd2b_w = qd2bs[it % 2]
                for hp in range(2):
                    for hs in range(2):
                        nc.sync.dma_start(
                            qd2b_w[64 * hp:64 * hp + 64,
                                   16 * hp + hs:16 * hp + 16:2, :],
                            stg[64 * hs:64 * hs + 64, :,
                                256 * hp:256 * hp + 256],
                        )

            def relayout_k(it, stg):
                """stg_k -> kht2[it%2]: kht2[64h'+d, q, pi] = k^T[d, q,
                row 256h'+pi]."""
                kht2_w = kht2s[it % 2]
                for hp in range(2):
                    for hs in range(2):
                        nc.sync.dma_start(
                            kht2_w[64 * hp:64 * hp + 64, hs:16:2, :],
                            stg[64 * hs:64 * hs + 64, :,
                                256 * hp:256 * hp + 256],
                        )

            def proj_v(it, xtc):
                """Natural (row-major) V projection, staged to DRAM with
                rows permuted to (w, b, B, m, h) order so the vd readback
                needs only 8 DMA instructions.  Chunk rc_ holds the rows
                with (h, B) = divmod(rc_, NBK); stage DMAs use plain
                partition slices (16 calls, on the Activation HWDGE)."""
                vstg = spool.tile([128, RC // 128, E], BF16, tag="stg_v")
                v3d = dpool.tile([2, 4, NBK, 16, 2, E], BF16, tag="dram_v")
                for rc_ in range(RC // 128):
                    for h2 in range(2):
                        ps = pproj.tile([128, 512], F32, tag="proj")
                        for c in range(8):
                            nc.tensor.matmul(
                                ps[:],
                                xtc[:, c, rc_ * 128:(rc_ + 1) * 128],
                                wv_sb[:, c, h2 * 512:(h2 + 1) * 512],
                                start=(c == 0),
                                stop=(c == 7),
                            )
                        if h2 == 0:
                            nc.vector.tensor_copy(vstg[:, rc_, 0:512], ps[:])
                        else:
                            nc.scalar.copy(vstg[:, rc_, 512:1024], ps[:])
                    h, B = divmod(rc_, NBK)
                    for w in range(2):
                        for b in range(4):
                            nc.sync.dma_start(
                                v3d[w, b, B, :, h, :],
                                vstg[64 * w + 16 * b:64 * w + 16 * b + 16,
                                     rc_, :],
                            )
                return v3d

            def vd_readback(it, v3d):
                """DRAM -> vd[it%2] block-diagonal V slabs: one DMA per
                (row-half w2, quarter b), 3-dim APs on both sides."""
                vd_w = vds[it % 2]
                for w2 in range(2):
                    for b in range(4):
                        nc.sync.dma_start(
                            vd_w[32 * b + 16 * w2:32 * b + 16 * w2 + 16,
                                 :, 64 * w2:64 * w2 + 64],
                            v3d[w2, b].rearrange(
                                "B m h (j d) -> j (B m h) d", j=16, d=64),
                        )

            def energy_softmax(B, par):
                """One dense 256-row energy bank + its softmax; returns att."""
                qd2b, kht2 = qd2bs[par], kht2s[par]
                ep = pe_pool.tile([128, 32, 16], F32, tag="ep")
                for lam in range(128):
                    pi = 128 * B + lam
                    b, s = (lam // 16) % 4, 2 * (lam % 16) + lam // 64
                    nc.tensor.matmul(
                        ep[32 * b:32 * b + 32, s, :],
                        qd2b[:, :, pi],
                        kht2[:, :, pi],
                        start=True,
                        stop=True,
                        tile_position=(0, 32 * b),
                    )
                ex = apool.tile([128, 32, 16], F32, tag="ex")
                nc.scalar.activation(ex[:], ep[:], AF.Exp)
                sm = apool.tile([128, 32], F32, tag="sm")
                nc.vector.reduce_sum(sm[:], ex[:], axis=AX.X)
                rcp = apool.tile([128, 32], F32, tag="rcp")
                nc.vector.reciprocal(rcp[:], sm[:])
                at = apool.tile([128, 32, 16], BF16, tag="at")
                nc.vector.tensor_tensor(
                    at[:], ex[:],
                    rcp[:, :, None].to_broadcast([128, 32, 16]),
                    ALU.mult,
                )
                att = apool.tile([128, 512], BF16, tag="att")
                nc.vector.transpose(att[:], at[:].rearrange("p a b -> p (a b)"))
                return att

            def av_extract(B, att, par):
                """A@V for bank B: two 2-bank psum tiles (b-halves) + 8
                merged extraction copies."""
                vd = vds[par]
                dstx = oft2[:].rearrange(
                    "p g (h Bk wc) -> p g h Bk wc", h=2, Bk=NBK)
                for bh in range(2):
                    avp = pav.tile([128, 2, 32, 16], F32, tag="avp")
                    for b2 in range(2):
                        b = 2 * bh + b2
                        for t in range(32):
                            nc.tensor.matmul(
                                avp[:, b2, t, :],
                                vd[32 * b:32 * b + 32, 32 * B + t, :],
                                att[32 * b:32 * b + 32, 16 * t:16 * t + 16],
                                start=True,
                                stop=True,
                                tile_position=(32 * b, 0),
                            )
                    # avp[64w+d, b2, 2m+rho, q] -> oft2[64(q%2)+d, q//2,
                    #   256B + 128rho + 64w + 32bh + 16b2 + m]
                    srcx = avp[:].rearrange(
                        "p b (m r) (g s) -> p g r (b m) s", r=2, s=2)
                    for w in range(2):
                        for sg in range(2):
                            srcc = srcx[64 * w:64 * w + 64, :, :, :, sg]
                            dst = dstx[64 * sg:64 * sg + 64, :, :, B,
                                       64 * w + 32 * bh:64 * w + 32 * bh + 32]
                            if (w + sg + B + bh) % 2 == 0:
                                nc.vector.tensor_copy(dst, srcc)
                            else:
                                nc.scalar.copy(dst, srcc)

            def wo_out(p):
                """y^T = Wo^T-chunks @ oft2, DMA out (bias added on host)."""
                for c in range(8):
                    # rotates through the ep buffers (attention is done
                    # with them by now) -> double-buffered Wo psum at no
                    # extra bank cost
                    ytp = pe_pool.tile([128, RC], F32, tag="ep")
                    for g in range(8):
                        nc.tensor.matmul(
                            ytp[:],
                            wo_sb[:, g, 128 * c:128 * c + 128],
                            oft2[:, g, :],
                            start=(g == 0),
                            stop=(g == 7),
                        )
                    ys = ypool.tile([128, RC], F32, tag="ys")
                    if c % 2 == 0:
                        nc.vector.tensor_copy(ys[:], ytp[:])
                    else:
                        nc.scalar.copy(ys[:], ytp[:])
                    nc.sync.dma_start(
                        yt.rearrange("(t q) r -> q t r", q=128)[
                            :, c, p * RC:(p + 1) * RC
                        ],
                        ys[:],
                    )

            # Depth-2 software pipeline: attention for pass p runs during
            # iteration p+1, interleaved with pass p+1's projections at
            # chunk granularity on the tensor queue.  All rebuild targets
            # (qd2b/kht2/vd) ping-pong on pass parity, so iteration it's
            # rebuilds (parity it%2) never touch the buffers pass it-1's
            # attention is reading (parity (it-1)%2).
            xtcs = {0: xtc0}
            for it in range(NP + 1):
                do_proj = it < NP
                do_att = it >= 1
                p = it - 1
                if do_proj:
                    xtc = xtcs.pop(it)
                    # prefetch next pass's x chunk a full iteration ahead
                    if it + 1 < NP:
                        r0 = (it + 1) * RC
                        xn = xpool.tile([128, 8, RC], BF16, tag="xtc")
                        nc.sync.dma_start(xn[:], xtr[:, :, r0:r0 + RC])
                        xtcs[it + 1] = xn

                # --- interleaved tensor-queue schedule (V first so its
                # DRAM round trip completes mid-iteration) ---
                if do_proj:
                    v3d = proj_v(it, xtc)
                    vd_readback(it, v3d)
                if do_att:
                    att0 = energy_softmax(0, p % 2)
                if do_proj:
                    q_stg = proj_tr("q", wq_sb, xtc)
                    relayout_q(it, q_stg)
                if do_att:
                    att1 = energy_softmax(1, p % 2)
                    av_extract(0, att0, p % 2)
                if do_proj:
                    k_stg = proj_tr("k", wk_sb, xtc)
                    relayout_k(it, k_stg)
                if do_att:
                    av_extract(1, att1, p % 2)
                    wo_out(p)

    nc.finalize()
    return nc


_CACHE = {}


def _get_nc(R, RC):
    key = (R, RC)
    if key not in _CACHE:
        _CACHE[key] = build_nc(R, RC)
    return _CACHE[key]


def run_cores(x2d, Wq, Wk, Wv, Wo, bo_v, R=None, RC=512, cores=None,
              **run_kwargs):
    """x2d: (ROWS, E) fp32.  Returns (ROWS, E) fp32."""
    ROWS = x2d.shape[0]
    if cores is None:
        cores = list(range(NCORE))
    n = len(cores)
    if R is None:
        R = ROWS // n
    assert R * n == ROWS
    nc = _get_nc(R, RC)

    bf = ml_dtypes.bfloat16
    scale = 1.0 / np.sqrt(np.sqrt(float(E)))  # fold E**-0.5 into both Wq, Wk
    wq_b = (Wq.astype(np.float64) * scale).astype(bf)
    wk_b = (Wk.astype(np.float64) * scale).astype(bf)
    wv_b = Wv.astype(bf)
    wo_b = Wo.astype(bf)
    bo_f = bo_v.reshape(1, E).astype(np.float32)

    in_maps = []
    for ci in range(n):
        xs = x2d[ci * R:(ci + 1) * R].T  # (E, R)
        in_maps.append({
            "xt": np.ascontiguousarray(xs).astype(bf),
            "wq": wq_b, "wk": wk_b, "wv": wv_b, "wo": wo_b,
        })
    res = run_bass_kernel_spmd(nc, in_maps, core_ids=cores, **run_kwargs)
    out = np.empty((ROWS, E), dtype=np.float32)
    for ci in range(n):
        ytd = res.results[ci]["yt"]  # (E, R), columns in natural row order
        out[ci * R:(ci + 1) * R] = ytd.T + bo_f  # bias added on host
    if run_kwargs.get("trace"):
        return out, res
    return out


def kernel(x, Wq, Wk, Wv, Wo, bo):
    x = np.asarray(x, dtype=np.float32)
    N, L, _ = x.shape
    y = run_cores(
        x.reshape(N * L, E),
        np.asarray(Wq, np.float32), np.asarray(Wk, np.float32),
        np.asarray(Wv, np.float32), np.asarray(Wo, np.float32),
        np.asarray(bo, np.float32),
    )
    return y.reshape(N, L, E)


# revision 28
# speedup vs baseline: 1.2131x; 1.0291x over previous
"""Trainium2 Bass kernel for the cross-head MultiHeadAttention module.

Reference computation (per row r of x flattened to (N*L, E)):
    q = x @ Wq; k = x @ Wk; v = x @ Wv           (E = 1024, H = 16, D = 64)
    energy[r, i, j] = sum_d q[r,i,d] * k[r,j,d]  (cross-head, per position)
    attn = softmax(energy / 32, axis=j)
    out[r, i, :] = sum_j attn[r,i,j] * v[r,j,:]
    y = out.reshape(R, E) @ Wo + bo

Distribution: data-parallel over rows (N*L = 16384 -> 2048 rows/core x 8).

Per-core design (all big matmuls in bf16 on the PE array), v5:
  *  Q/K projections run transposed (features on partitions, rows free).
     Their attention-layout rebuild (qd2b/kht2 block-diagonal slabs) is a
     pure partition-shift, so it runs as 32 direct SBUF->SBUF DMAs per
     tensor (512B runs both sides) with NO DRAM round trip; each DMA only
     depends on one projection chunk's psum->sbuf copy, so the rebuild
     streams behind the projection instead of after it.
  *  V runs natural (rows on partitions) and still round-trips through
     DRAM (its rebuild needs a partition<->free exchange which SBUF APs
     cannot express).  vd is double-buffered by pass parity so the
     readback issues immediately after the stage-out instead of waiting
     for the previous pass's AV matmuls (WAR removed).
  *  Energy: ONE matmul per row pair (pi, pi+RC/2): lhsT = qd2b[:, :, pi]
     ([128, 32] block-diagonal), rhs = kht2[:, :, pi] ([128, 16]); out is
     a dense [32, 16] block of a 256-row psum bank so softmax runs on
     dense [128, 512] tiles.
  *  softmax: exp (no max-subtraction: energies ~N(0, 1/16)), row-sum,
     reciprocal, scale+cast-to-bf16, 32x32-block vector transpose.
  *  A@V: ONE matmul per row pair; avp psum is 2 double-buffered 2-bank
     tiles (b-halves) so bank1's matmuls only wait on bank0's first-half
     extraction.
  *  y^T: full-width Wo matmuls accumulated in psum; + bo; DMA out.
  *  Schedule: the tensor queue interleaves pass p-1's attention with
     pass p's projections at chunk granularity:
       projq(p) | energy0(p-1) | projk(p) | energy1(p-1) | av0(p-1) |
       projv(p)+vd-readback(p) | av1(p-1) | Wo(p-1)
     so softmax / extraction / readback chains hide under projection
     matmuls and the PE p-state stays high.  Startup DMAs are split
     per-128-column chunk and ordered xtc -> Wq -> Wk -> Wv -> Wo.
"""

import numpy as np
import ml_dtypes

import concourse.bass as bass
from concourse import bacc
import concourse.tile as tile
from concourse import mybir
from concourse.bass_utils import run_bass_kernel_spmd

F32 = mybir.dt.float32
BF16 = mybir.dt.bfloat16
AF = mybir.ActivationFunctionType
ALU = mybir.AluOpType
AX = mybir.AxisListType

E = 1024
H = 16
D = 64
NCORE = 8


def build_nc(R, RC):
    """Per-core kernel program: R rows total, processed in passes of RC."""
    NP = R // RC          # passes
    NBK = RC // 256       # dense energy banks per pass (256 rows each)
    PH = RC // 2          # row pairs per pass

    nc = bacc.Bacc("TRN2", target_bir_lowering=False, debug=False)

    xt = nc.dram_tensor("xt", [E, R], BF16, kind="ExternalInput")
    wq = nc.dram_tensor("wq", [E, E], BF16, kind="ExternalInput")
    wk = nc.dram_tensor("wk", [E, E], BF16, kind="ExternalInput")
    wv = nc.dram_tensor("wv", [E, E], BF16, kind="ExternalInput")
    wo = nc.dram_tensor("wo", [E, E], BF16, kind="ExternalInput")
    yt = nc.dram_tensor("yt", [E, R], BF16, kind="ExternalOutput")

    with tile.TileContext(nc) as tc:
        with (
            tc.tile_pool(name="wpool", bufs=1) as wpool,      # persistent
            tc.tile_pool(name="xpool", bufs=2) as xpool,      # xt chunks
            tc.tile_pool(name="spool", bufs=1) as spool,      # q/k/v staging
            tc.tile_pool(name="apool", bufs=2) as apool,      # softmax temps
            tc.tile_pool(name="opool", bufs=1) as opool,      # oft2
            tc.tile_pool(name="ypool", bufs=2) as ypool,      # y staging
            tc.tile_pool(name="dram", bufs=2, space="DRAM") as dpool,
            tc.tile_pool(name="pproj", bufs=2, space="PSUM") as pproj,
            tc.tile_pool(name="pe", bufs=2, space="PSUM") as pe_pool,
            tc.tile_pool(name="pav", bufs=2, space="PSUM") as pav,
        ):
            # ---- persistent loads (split per 128-col chunk; xtc(0) + wq
            # first so the first projection starts as early as possible).
            # DMA traffic classes: latency-critical relayouts (qk, vd) +
            # xtc go on the SP HWDGE (nc.sync); bulk streams (weights,
            # V stage-out, y out) go on the Activation HWDGE (nc.scalar)
            # so bulk descriptor storms never sit ahead of critical ones.
            wq_sb = wpool.tile([128, 8, E], BF16, tag="wq")
            wk_sb = wpool.tile([128, 8, E], BF16, tag="wk")
            wv_sb = wpool.tile([128, 8, E], BF16, tag="wv")
            wo_sb = wpool.tile([128, 8, E], BF16, tag="wo")

            xtr = xt.rearrange("(c p) r -> p c r", p=128)
            xtc0 = xpool.tile([128, 8, RC], BF16, tag="xtc")
            for c in range(8):
                nc.sync.dma_start(xtc0[:, c, :], xtr[:, c, 0:RC])
            wvr = wv.rearrange("(c p) e -> p c e", p=128)
            for c in range(8):
                nc.sync.dma_start(wv_sb[:, c, :], wvr[:, c, :])
            wqr = wq.rearrange("(c p) e -> p c e", p=128)
            for c in range(8):
                nc.sync.dma_start(wq_sb[:, c, :], wqr[:, c, :])
            nc.sync.dma_start(
                wk_sb[:], wk.rearrange("(c p) e -> p c e", p=128))
            nc.sync.dma_start(
                wo_sb[:], wo.rearrange("(c p) e -> p c e", p=128))

            # block-diagonal operand tiles: zero blocks are memset once and
            # never rewritten (per-pass DMAs touch only the data blocks).
            # qd2b/kht2/vd ping-pong per pass parity so pass p+1's rebuild
            # DMAs never wait on pass p's attention matmuls.
            qd2bs, kht2s, vds = [], [], []
            for pp in range(2):
                qd2b_ = wpool.tile([128, 32, PH], BF16, tag=f"qd2b{pp}",
                                   name=f"qd2b{pp}")
                nc.gpsimd.memset(qd2b_[0:64, 16:32, :], 0.0)
                nc.gpsimd.memset(qd2b_[64:128, 0:16, :], 0.0)
                qd2bs.append(qd2b_)
                kht2_ = wpool.tile([128, 16, PH], BF16, tag=f"kht2{pp}",
                                   name=f"kht2{pp}")
                kht2s.append(kht2_)
                vd_ = wpool.tile([128, NBK * 32, 128], BF16, tag=f"vd{pp}",
                                 name=f"vd{pp}")
                nc.gpsimd.memset(vd_[:], 0.0)
                vds.append(vd_)

            oft2 = opool.tile([128, 8, RC], BF16, tag="oft2")

            def proj_tr(name, w_sb, xtc):
                """Transposed projection (features on partitions)."""
                stg = spool.tile([128, 8, RC], BF16, tag=f"stg_{name}")
                for et in range(8):
                    ps = pproj.tile([128, RC], F32, tag="proj")
                    for c in range(8):
                        nc.tensor.matmul(
                            ps[:],
                            w_sb[:, c, et * 128:(et + 1) * 128],
                            xtc[:, c, :],
                            start=(c == 0),
                            stop=(c == 7),
                        )
                    if et % 3 < 2:
                        nc.vector.tensor_copy(stg[:, et, :], ps[:])
                    else:
                        nc.scalar.copy(stg[:, et, :], ps[:])
                return stg

            def relayout_q(it, stg):
                """stg_q -> qd2b[it%2] via 4 partition-shift SBUF DMAs.

                qd2b[64h'+d, 16h'+q, pi] = q^T[d, head q, row 256h'+pi]
                and stg[64(q%2)+d, q//2, r] = q^T[d, head q, row r], so
                each (h', q%2) group of 8 heads is one strided DMA.
                """
                qd2b_w = qd2bs[it % 2]
                for hp in range(2):
                    for hs in range(2):
                        nc.sync.dma_start(
                            qd2b_w[64 * hp:64 * hp + 64,
                                   16 * hp + hs:16 * hp + 16:2, :],
                            stg[64 * hs:64 * hs + 64, :,
                                256 * hp:256 * hp + 256],
                        )

            def relayout_k(it, stg):
                """stg_k -> kht2[it%2]: kht2[64h'+d, q, pi] = k^T[d, q,
                row 256h'+pi]."""
                kht2_w = kht2s[it % 2]
                for hp in range(2):
                    for hs in range(2):
                        nc.sync.dma_start(
                            kht2_w[64 * hp:64 * hp + 64, hs:16:2, :],
                            stg[64 * hs:64 * hs + 64, :,
                                256 * hp:256 * hp + 256],
                        )

            def proj_v(it, xtc):
                """Natural (row-major) V projection, staged to DRAM with
                rows permuted to (w, b, B, m, h) order so the vd readback
                needs only 8 DMA instructions.  Chunk rc_ holds the rows
                with (h, B) = divmod(rc_, NBK); stage DMAs use plain
                partition slices (16 calls, on the Activation HWDGE)."""
                vstg = spool.tile([128, RC // 128, E], BF16, tag="stg_v")
                v3d = dpool.tile([2, 4, NBK, 16, 2, E], BF16, tag="dram_v")
                for rc_ in range(RC // 128):
                    for h2 in range(2):
                        ps = pproj.tile([128, 512], F32, tag="proj")
                        for c in range(8):
                            nc.tensor.matmul(
                                ps[:],
                                xtc[:, c, rc_ * 128:(rc_ + 1) * 128],
                                wv_sb[:, c, h2 * 512:(h2 + 1) * 512],
                                start=(c == 0),
                                stop=(c == 7),
                            )
                        if h2 == 0:
                            nc.vector.tensor_copy(vstg[:, rc_, 0:512], ps[:])
                        else:
                            nc.scalar.copy(vstg[:, rc_, 512:1024], ps[:])
                    h, B = divmod(rc_, NBK)
                    for w in range(2):
                        for b in range(4):
                            nc.sync.dma_start(
                                v3d[w, b, B, :, h, :],
                                vstg[64 * w + 16 * b:64 * w + 16 * b + 16,
                                     rc_, :],
                            )
                return v3d

            def vd_readback(it, v3d):
                """DRAM -> vd[it%2] block-diagonal V slabs: one DMA per
                (row-half w2, quarter b), 3-dim APs on both sides."""
                vd_w = vds[it % 2]
                for w2 in range(2):
                    for b in range(4):
                        nc.sync.dma_start(
                            vd_w[32 * b + 16 * w2:32 * b + 16 * w2 + 16,
                                 :, 64 * w2:64 * w2 + 64],
                            v3d[w2, b].rearrange(
                                "B m h (j d) -> j (B m h) d", j=16, d=64),
                        )

            def energy_softmax(B, par):
                """One dense 256-row energy bank + its softmax; returns att."""
                qd2b, kht2 = qd2bs[par], kht2s[par]
                ep = pe_pool.tile([128, 32, 16], F32, tag="ep")
                for lam in range(128):
                    pi = 128 * B + lam
                    b, s = (lam // 16) % 4, 2 * (lam % 16) + lam // 64
                    nc.tensor.matmul(
                        ep[32 * b:32 * b + 32, s, :],
                        qd2b[:, :, pi],
                        kht2[:, :, pi],
                        start=True,
                        stop=True,
                        tile_position=(0, 32 * b),
                    )
                ex = apool.tile([128, 32, 16], F32, tag="ex")
                nc.scalar.activation(ex[:], ep[:], AF.Exp)
                sm = apool.tile([128, 32], F32, tag="sm")
                nc.vector.reduce_sum(sm[:], ex[:], axis=AX.X)
                rcp = apool.tile([128, 32], F32, tag="rcp")
                nc.vector.reciprocal(rcp[:], sm[:])
                at = apool.tile([128, 32, 16], BF16, tag="at")
                nc.vector.tensor_tensor(
                    at[:], ex[:],
                    rcp[:, :, None].to_broadcast([128, 32, 16]),
                    ALU.mult,
                )
                att = apool.tile([128, 512], BF16, tag="att")
                nc.vector.transpose(att[:], at[:].rearrange("p a b -> p (a b)"))
                return att

            def av_extract(B, att, par):
                """A@V for bank B: two 2-bank psum tiles (b-halves) + 8
                merged extraction copies."""
                vd = vds[par]
                dstx = oft2[:].rearrange(
                    "p g (h Bk wc) -> p g h Bk wc", h=2, Bk=NBK)
                for bh in range(2):
                    avp = pav.tile([128, 2, 32, 16], F32, tag="avp")
                    for b2 in range(2):
                        b = 2 * bh + b2
                        for t in range(32):
                            nc.tensor.matmul(
                                avp[:, b2, t, :],
                                vd[32 * b:32 * b + 32, 32 * B + t, :],
                                att[32 * b:32 * b + 32, 16 * t:16 * t + 16],
                                start=True,
                                stop=True,
                                tile_position=(32 * b, 0),
                            )
                    # avp[64w+d, b2, 2m+rho, q] -> oft2[64(q%2)+d, q//2,
                    #   256B + 128rho + 64w + 32bh + 16b2 + m]
                    srcx = avp[:].rearrange(
                        "p b (m r) (g s) -> p g r (b m) s", r=2, s=2)
                    for w in range(2):
                        for sg in range(2):
                            srcc = srcx[64 * w:64 * w + 64, :, :, :, sg]
                            dst = dstx[64 * sg:64 * sg + 64, :, :, B,
                                       64 * w + 32 * bh:64 * w + 32 * bh + 32]
                            if (w + sg + B + bh) % 2 == 0:
                                nc.vector.tensor_copy(dst, srcc)
                            else:
                                nc.scalar.copy(dst, srcc)

            def wo_out(p):
                """y^T = Wo^T-chunks @ oft2, DMA out (bias added on host)."""
                for c in range(8):
                    # rotates through the ep buffers (attention is done
                    # with them by now) -> double-buffered Wo psum at no
                    # extra bank cost
                    ytp = pe_pool.tile([128, RC], F32, tag="ep")
                    for g in range(8):
                        nc.tensor.matmul(
                            ytp[:],
                            wo_sb[:, g, 128 * c:128 * c + 128],
                            oft2[:, g, :],
                            start=(g == 0),
                            stop=(g == 7),
                        )
                    ys = ypool.tile([128, RC], BF16, tag="ys")
                    if c % 2 == 0:
                        nc.vector.tensor_copy(ys[:], ytp[:])
                    else:
                        nc.scalar.copy(ys[:], ytp[:])
                    nc.sync.dma_start(
                        yt.rearrange("(t q) r -> q t r", q=128)[
                            :, c, p * RC:(p + 1) * RC
                        ],
                        ys[:],
                    )

            # Depth-2 software pipeline: attention for pass p runs during
            # iteration p+1, interleaved with pass p+1's projections at
            # chunk granularity on the tensor queue.  All rebuild targets
            # (qd2b/kht2/vd) ping-pong on pass parity, so iteration it's
            # rebuilds (parity it%2) never touch the buffers pass it-1's
            # attention is reading (parity (it-1)%2).
            xtcs = {0: xtc0}
            for it in range(NP + 1):
                do_proj = it < NP
                do_att = it >= 1
                p = it - 1
                if do_proj:
                    xtc = xtcs.pop(it)
                    # prefetch next pass's x chunk a full iteration ahead
                    if it + 1 < NP:
                        r0 = (it + 1) * RC
                        xn = xpool.tile([128, 8, RC], BF16, tag="xtc")
                        nc.sync.dma_start(xn[:], xtr[:, :, r0:r0 + RC])
                        xtcs[it + 1] = xn

                # --- interleaved tensor-queue schedule (V first so its
                # DRAM round trip completes mid-iteration) ---
                if do_proj:
                    v3d = proj_v(it, xtc)
                    vd_readback(it, v3d)
                if do_att:
                    att0 = energy_softmax(0, p % 2)
                if do_proj:
                    q_stg = proj_tr("q", wq_sb, xtc)
                    relayout_q(it, q_stg)
                if do_att:
                    att1 = energy_softmax(1, p % 2)
                    av_extract(0, att0, p % 2)
                if do_proj:
                    k_stg = proj_tr("k", wk_sb, xtc)
                    relayout_k(it, k_stg)
                if do_att:
                    av_extract(1, att1, p % 2)
                    wo_out(p)

    nc.finalize()
    return nc


_CACHE = {}


def _get_nc(R, RC):
    key = (R, RC)
    if key not in _CACHE:
        _CACHE[key] = build_nc(R, RC)
    return _CACHE[key]


def run_cores(x2d, Wq, Wk, Wv, Wo, bo_v, R=None, RC=512, cores=None,
              **run_kwargs):
    """x2d: (ROWS, E) fp32.  Returns (ROWS, E) fp32."""
    ROWS = x2d.shape[0]
    if cores is None:
        cores = list(range(NCORE))
    n = len(cores)
    if R is None:
        R = ROWS // n
    assert R * n == ROWS
    nc = _get_nc(R, RC)

    bf = ml_dtypes.bfloat16
    scale = 1.0 / np.sqrt(np.sqrt(float(E)))  # fold E**-0.5 into both Wq, Wk
    wq_b = (Wq.astype(np.float64) * scale).astype(bf)
    wk_b = (Wk.astype(np.float64) * scale).astype(bf)
    wv_b = Wv.astype(bf)
    wo_b = Wo.astype(bf)
    bo_f = bo_v.reshape(1, E).astype(np.float32)

    in_maps = []
    for ci in range(n):
        xs = x2d[ci * R:(ci + 1) * R].T  # (E, R)
        in_maps.append({
            "xt": np.ascontiguousarray(xs).astype(bf),
            "wq": wq_b, "wk": wk_b, "wv": wv_b, "wo": wo_b,
        })
    res = run_bass_kernel_spmd(nc, in_maps, core_ids=cores, **run_kwargs)
    out = np.empty((ROWS, E), dtype=np.float32)
    for ci in range(n):
        ytd = res.results[ci]["yt"]  # (E, R), columns in natural row order
        out[ci * R:(ci + 1) * R] = ytd.T + bo_f  # bias added on host
    if run_kwargs.get("trace"):
        return out, res
    return out


def kernel(x, Wq, Wk, Wv, Wo, bo):
    x = np.asarray(x, dtype=np.float32)
    N, L, _ = x.shape
    y = run_cores(
        x.reshape(N * L, E),
        np.asarray(Wq, np.float32), np.asarray(Wk, np.float32),
        np.asarray(Wv, np.float32), np.asarray(Wo, np.float32),
        np.asarray(bo, np.float32),
    )
    return y.reshape(N, L, E)
